# revision 60
# baseline (speedup 1.0000x reference)
"""Causal self-attention (B=4, T=2048, C=1024, H=16, D=64) on 8 TRN2 NeuronCores.

Sharding: core = 2*b + g  (b = batch 0..3, g = head-group 0..1; heads 8g..8g+7).
Each core computes, for its batch b and its 8 heads:
  qkv projection, causal softmax attention, and a PARTIAL output projection
  (its 512 rows of W_proj). Host sums the two partials per batch (+ b_proj).

Final design (301.7us baseline -> 262.6us; bf16 matmuls, f32 PSUM accum):
  - all matmul operands bf16: 1 cycle/row on PE at ANY free size (fp32r pays
    4x below 256 cols), halved DMA and SBUF footprint. bf16 noise ~3e-3 rel
    vs the 2e-2 budget (fp8 would blow it: ~5% q/k error -> ~0.2 abs).
  - softmax denominator for free: AV lhsT = [V_h | ones64] (128 cols), so
    PSUM rows 64:127 = sum_k P replicated across 64 partitions. Normalize =
    DVE reciprocal (partition-aligned 64:128) + ACT Identity copy of y,
    then SBUF->SBUF DMA shifts 1/den down to partitions 0:63 (engines are
    partition-locked; only DMA/PE can move data across partitions) + DVE
    multiply. Only ONE psum operand per vector op is legal.
  - consolidated DMAs: transfers run FIFO in descriptor order at ~350 B/ns
    with ~625ns serial descriptor gen per queue, so emission order IS the
    transfer schedule; front-load exactly what the V phase consumes first.
  - chunk-outer schedule with a filler queue: V(t0..7); {qk(p,ch0);
    att(p,c0) + V(t8..15) fillers} x4; {qk(p,ch1); att(p,c1) + out-proj
    qtile 0..7 fillers} x4; out-proj 8..15. The exp stream on ACT
    (0.83ns/col, ~152us) is the attention-phase co-bottleneck; popping one
    independent PE op between S(ki) and AV(ki-1) keeps PE fed through exp
    latency. Fillers and direct emitters share the "small" psum tag, so the
    queue must fully drain before any direct allocation (buffer rotation
    would corrupt an in-flight accumulation).
  - 1-deep software pipeline inside a head-chunk: emit S(ki+1) before AV(ki).
  - PE warmup: throwaway matmuls during the initial DMA wait ramp the PE
    p-state (1.54 -> 0.42 ns/col) before real work arrives.
  - PSUM banks (8): stp [128,1024] bufs=2 (4) + ytps [128,1024] (2) +
    small [128,512] bufs=2 (2).
  - psum->SBUF copy engines chosen to keep FIFOs clear: qk q-side DVE,
    k-side ACT Identity (per-partition bias AP); out-proj tail alternates
    DVE/ACT; V copies DVE; v-ones memsets per-tile on DVE (strided).
"""

import sys

try:
    import concourse  # noqa: F401
except ImportError:
    sys.path.insert(0, "/opt/trn_rl_repo")

import numpy as np
import ml_dtypes

import concourse.bacc as bacc
import concourse.mybir as mybir
import concourse.tile as tile

F32 = mybir.dt.float32
BF16 = mybir.dt.bfloat16
AF = mybir.ActivationFunctionType
ALU = mybir.AluOpType

B, T, C = 4, 2048, 1024
H, D = 16, 64
NCORES = 8
HL = 8          # heads per core (local)
NPAIR = 4       # head pairs per core
CH = 1024       # query chunk
NCH = T // CH   # 2
KT = T // 128   # 16 key tiles
CT = C // 128   # 8 contraction tiles over C
SCALE = 1.0 / 8.0  # 1/sqrt(D)

NPBF16 = ml_dtypes.bfloat16

_prog_cache = {}


def build_program(debug=False):
    key = debug
    if key in _prog_cache:
        return _prog_cache[key]

    nc = bacc.Bacc(None, target_bir_lowering=False, debug=debug)

    xt = nc.dram_tensor("xt", [C, T], BF16, kind="ExternalInput")
    wq = nc.dram_tensor("wq", [C, 512], BF16, kind="ExternalInput")
    wk = nc.dram_tensor("wk", [C, 512], BF16, kind="ExternalInput")
    wv = nc.dram_tensor("wv", [C, 512], BF16, kind="ExternalInput")
    bqk_t = nc.dram_tensor("bqk_t", [128, 8], F32, kind="ExternalInput")
    bv = nc.dram_tensor("bv", [1, 512], BF16, kind="ExternalInput")
    wp = nc.dram_tensor("wp", [512, C], BF16, kind="ExternalInput")
    out = nc.dram_tensor("out", [T, C], BF16, kind="ExternalOutput")

    with tile.TileContext(nc) as tc:
        with (
            tc.tile_pool(name="consts", bufs=1) as consts,
            tc.tile_pool(name="xtp", bufs=1) as xtp,
            tc.tile_pool(name="wvp", bufs=1) as wvp,
            tc.tile_pool(name="w8p", bufs=1) as w8p,
            tc.tile_pool(name="wpp", bufs=1) as wpp,
            tc.tile_pool(name="vp", bufs=1) as vp,
            tc.tile_pool(name="qkp", bufs=1) as qkp,
            tc.tile_pool(name="ytp", bufs=1) as ytp,
            tc.tile_pool(name="ptp", bufs=6) as ptp,
            tc.tile_pool(name="rcpp", bufs=2) as rcpp,
            tc.tile_pool(name="ytup", bufs=2) as ytup,
            tc.tile_pool(name="outp", bufs=3) as outp,
            tc.tile_pool(name="ps", bufs=1, space="PSUM") as ps,
        ):
            # ================= DMA staging (emission order = priority) ======
            # Transfers run FIFO in descriptor order on the shared DMA-engine
            # pool (~350 B/ns aggregate) and descriptor gen is ~625ns serial
            # per queue — so emission order IS the transfer schedule. One SP
            # chain, ordered by when the compute consumes each piece.
            vecs = consts.tile([128, 640], BF16, tag="vecs")
            bv_sb = vecs[32:33, 0:512]
            nc.sync.dma_start(out=bv_sb, in_=bv[:, :])
            xt_sb = xtp.tile([128, CT, T], BF16, tag="xt")
            xt_r = xt.ap().rearrange("(k p) t -> p k t", p=128)
            nc.sync.dma_start(out=xt_sb[:, :, 0:128], in_=xt_r[:, :, 0:128])
            wv_sb = wvp.tile([128, CT, 512], BF16, tag="wv")
            wv_r = wv.ap().rearrange("(k p) n -> p k n", p=128)
            for (k0_, k1_) in ((0, 1), (1, 4), (4, CT)):
                nc.sync.dma_start(
                    out=wv_sb[:, k0_:k1_, :], in_=wv_r[:, k0_:k1_, :]
                )
            for (c0_, c1_) in ((128, 256), (256, 512), (512, 1024)):
                nc.sync.dma_start(
                    out=xt_sb[:, :, c0_:c1_], in_=xt_r[:, :, c0_:c1_]
                )

            bqk_sb = consts.tile([128, 8], F32, tag="bqk")
            nc.sync.dma_start(out=bqk_sb, in_=bqk_t[:, :])

            # qk weights: w8[(p, side)] = [128, CT, 128] bf16 (one desc each)
            w8_sb = {}
            for p in range(NPAIR):
                for side, wsrc in ((0, wq), (1, wk)):
                    t_ = w8p.tile([128, CT, 128], BF16, tag=f"w8_{p}_{side}")
                    w_src = wsrc.ap().rearrange(
                        "(k pp) m -> pp k m", pp=128
                    )[:, :, p * 128:(p + 1) * 128]
                    nc.sync.dma_start(out=t_, in_=w_src)
                    w8_sb[(p, side)] = t_

            # wp: one descriptor, [128, NPAIR, 1024]
            wp_sb = wpp.tile([128, NPAIR, C], BF16, tag="wp")
            nc.sync.dma_start(
                out=wp_sb, in_=wp.ap().rearrange("(k p) n -> p k n", p=128)
            )
            # xt tail columns — only needed from V(t8..15) / qk ch1 onward
            for (c0_, c1_) in ((1024, 1536), (1536, 2048)):
                nc.sync.dma_start(
                    out=xt_sb[:, :, c0_:c1_], in_=xt_r[:, :, c0_:c1_]
                )

            # constants
            ones_f32 = consts.tile([128, 128], F32, tag="ones_f32")
            nc.vector.memset(ones_f32, 1.0)
            triu_f32 = consts.tile([128, 128], F32, tag="triu_f32")
            nc.gpsimd.memset(triu_f32, 1.0)
            nc.gpsimd.affine_select(
                out=triu_f32, in_=triu_f32,
                compare_op=ALU.is_ge,
                fill=0.0, base=0, pattern=[[1, 128]], channel_multiplier=-1,
            )
            triu_sb = consts.tile([128, 128], BF16, tag="triu")
            nc.vector.tensor_copy(triu_sb, triu_f32)
            # rank-1 v-bias vector ones: lhsT/rhs share base partition 32
            ones32_sb = vecs[32:33, 512:640]
            nc.vector.tensor_copy(ones32_sb, ones_f32[0:1, 0:128])

            # ================= persistent SBUF state ========================
            # v tiles: [128, 8 heads x (64 v-dims | 64 ones)]
            v_sb = []
            for t in range(KT):
                t_ = vp.tile([128, HL * 128], BF16, tag=f"v{t}", name=f"v{t}")
                v_sb.append(t_)

            def memset_v_ones(t):
                # ones columns only (strided): DVE, emitted per-tile right
                # before its V projection so 16 memsets don't jam the DVE
                # FIFO ahead of the V-phase psum copies. (gpsimd.memset on a
                # bf16 tile produced wrong bits on HW — keep this on DVE.)
                v_r = v_sb[t].rearrange("p (h x) -> p h x", h=HL)
                nc.vector.memset(v_r[:, :, 64:128], 1.0)
            qt_sb = [qkp.tile([128, T], BF16, tag=f"qt{p}", name=f"qt{p}")
                     for p in range(NPAIR)]
            kt_sb = [qkp.tile([128, T], BF16, tag=f"kt{p}", name=f"kt{p}")
                     for p in range(NPAIR)]
            yt_sb = [ytp.tile([128, T], BF16, tag=f"yt{p}", name=f"yt{p}")
                     for p in range(NPAIR)]

            # ---- PE warmup: the tensor engine ramps (1.54 -> 0.83 -> 0.42
            # ns/col over ~3us of continuous work). Burn the initial DMA wait
            # on throwaway matmuls over the freshly-memset v tile so the real
            # V projection starts at full clock.
            warm = ps.tile([128, CH], F32, tag="stp", bufs=2, name="warm")
            for i_ in range(7):
                s0 = 512 * (i_ % 2)
                nc.tensor.matmul(
                    warm[:, s0:s0 + 512],
                    lhsT=v_sb[0][:, 0:128], rhs=v_sb[0][:, 0:512],
                    start=True, stop=True,
                )

            # ================= filler queue =================================
            # The exp stream on ACT runs ~0.2us/ki slower than PE's S+AV, so
            # stalls accumulate inside a head-chunk. Queue independent PE work
            # (V tiles, out-proj qtiles) as single-matmul callables and pop
            # one between S(ki) and AV(ki-1) — PE chews filler exactly where
            # it would otherwise wait for exp(ki-1).
            from collections import deque
            fill_q = deque()

            def fill(n=1):
                for _ in range(n):
                    if not fill_q:
                        return
                    fill_q.popleft()()

            def drain_all():
                """MUST run before any direct 'small'/'ot' tile allocation:
                a queued unit left half-emitted would have its rotating psum
                buffer stolen mid-accumulation (silent corruption)."""
                while fill_q:
                    fill(1)

            def queue_v(t):
                """Enqueue V projection for key-tile t as per-op callables."""
                st = {}

                def mk_mm(k):
                    def f():
                        if k == 0:
                            st["pv"] = ps.tile([128, 512], F32, tag="small",
                                               bufs=2, name=f"pv{t}")
                        nc.tensor.matmul(
                            st["pv"],
                            lhsT=xt_sb[:, k, t * 128:(t + 1) * 128],
                            rhs=wv_sb[:, k, :], start=(k == 0), stop=False,
                        )
                    return f

                def bias():
                    nc.tensor.matmul(
                        st["pv"], lhsT=ones32_sb, rhs=bv_sb,
                        start=False, stop=True,
                    )

                def copy():
                    v_r = v_sb[t].rearrange("p (h x) -> p h x", h=HL)
                    pv_r = st["pv"].rearrange("p (h d) -> p h d", h=HL)
                    nc.vector.tensor_copy(v_r[:, :, 0:64], pv_r)

                fill_q.append(lambda: memset_v_ones(t))
                for k in range(CT):
                    fill_q.append(mk_mm(k))
                fill_q.append(bias)
                fill_q.append(copy)

            def queue_qk(p, ch):
                """Enqueue qk projection (filler variant: copies on DVE only,
                so no exp-stream interference when popped mid-attention).
                Returns a counter dict; drain until counter hits 0 before
                emitting anything that reads qt/kt of this pair+chunk."""
                st = {}
                cnt = {"n": 0}

                def wrap(f):
                    cnt["n"] += 1

                    def g():
                        f()
                        cnt["n"] -= 1
                    return g

                def mk_mm(side, s0, k):
                    def f():
                        if k == 0:
                            st[(side, s0)] = ps.tile(
                                [128, 512], F32, tag="small", bufs=2,
                                name=f"pq{p}_{side}_{ch}_{s0}")
                        nc.tensor.matmul(
                            st[(side, s0)],
                            lhsT=w8_sb[(p, side)][:, k, :],
                            rhs=xt_sb[:, k, ch * CH + s0:ch * CH + s0 + 512],
                            start=(k == 0), stop=(k == CT - 1),
                        )
                    return f

                def mk_copy(side, s0):
                    dst = qt_sb[p] if side == 0 else kt_sb[p]
                    bcol = bqk_sb[:, 4 * side + p:4 * side + p + 1]

                    def f():
                        nc.vector.tensor_scalar(
                            out=dst[:, ch * CH + s0:ch * CH + s0 + 512],
                            in0=st[(side, s0)], scalar1=bcol, scalar2=None,
                            op0=ALU.add,
                        )
                    return f

                for side in (0, 1):
                    for s0 in (0, 512):
                        for k in range(CT):
                            fill_q.append(wrap(mk_mm(side, s0, k)))
                        fill_q.append(wrap(mk_copy(side, s0)))
                return cnt

            def queue_out(qt_i):
                """Enqueue out-projection for query tile qt_i (DVE copies)."""
                st = {}

                def mk_mm(s0, p):
                    def f():
                        if p == 0:
                            st[s0] = ps.tile([128, 512], F32, tag="small",
                                             bufs=2, name=f"pso{qt_i}_{s0}")
                        nc.tensor.matmul(
                            st[s0],
                            lhsT=yt_sb[p][:, qt_i * 128:(qt_i + 1) * 128],
                            rhs=wp_sb[:, p, s0:s0 + 512],
                            start=(p == 0), stop=(p == NPAIR - 1),
                        )
                    return f

                def mk_copy(s0):
                    def f():
                        if "ot" not in st:
                            st["ot"] = outp.tile([128, C], BF16, tag="ot",
                                                 name=f"ot{qt_i}")
                        nc.vector.tensor_copy(
                            st["ot"][:, s0:s0 + 512], st[s0]
                        )
                    return f

                def dma():
                    nc.sync.dma_start(
                        out=out.ap()[qt_i * 128:(qt_i + 1) * 128, :],
                        in_=st["ot"],
                    )

                for s0 in (0, 512):
                    for p in range(NPAIR):
                        fill_q.append(mk_mm(s0, p))
                    fill_q.append(mk_copy(s0))
                fill_q.append(dma)

            # ================= phase emitters ===============================
            def emit_v(t):
                """V projection for key-tile t -> v_sb[t]."""
                memset_v_ones(t)
                pv = ps.tile([128, 512], F32, tag="small", bufs=2,
                             name=f"pv{t}")
                for k in range(CT):
                    nc.tensor.matmul(
                        pv,
                        lhsT=xt_sb[:, k, t * 128:(t + 1) * 128],
                        rhs=wv_sb[:, k, :], start=(k == 0), stop=False,
                    )
                nc.tensor.matmul(
                    pv, lhsT=ones32_sb, rhs=bv_sb,
                    start=False, stop=True,
                )
                v_r = v_sb[t].rearrange("p (h x) -> p h x", h=HL)
                pv_r = pv.rearrange("p (h d) -> p h d", h=HL)
                nc.vector.tensor_copy(v_r[:, :, 0:64], pv_r)

            def emit_qk(p, ch):
                """qk projection for pair p, T-chunk ch -> qt/kt cols."""
                drain_all()
                for side, dst in ((0, qt_sb[p]), (1, kt_sb[p])):
                    w8 = w8_sb[(p, side)]
                    bcol = bqk_sb[:, 4 * side + p:4 * side + p + 1]
                    for s0 in (0, 512):
                        pq = ps.tile([128, 512], F32, tag="small", bufs=2,
                                     name=f"pq{p}_{side}_{ch}_{s0}")
                        for k in range(CT):
                            nc.tensor.matmul(
                                pq,
                                lhsT=w8[:, k, :],
                                rhs=xt_sb[:, k,
                                          ch * CH + s0:ch * CH + s0 + 512],
                                start=(k == 0), stop=(k == CT - 1),
                            )
                        # q-side copies on DVE, k-side on ACT (GPSIMD cannot
                        # read PSUM): the first S matmul needs BOTH qt and kt
                        # — parallel engines halve that critical-path latency.
                        if side == 0:
                            nc.vector.tensor_scalar(
                                out=dst[:, ch * CH + s0:ch * CH + s0 + 512],
                                in0=pq, scalar1=bcol, scalar2=None,
                                op0=ALU.add,
                            )
                        else:
                            nc.scalar.activation(
                                out=dst[:, ch * CH + s0:ch * CH + s0 + 512],
                                in_=pq, func=AF.Identity, bias=bcol, scale=1.0,
                            )

            def emit_att(p, c):
                """Attention for pair p's two heads over query chunk c."""
                kmax = 8 * (c + 1)
                for hh in range(2):
                    hloc = 2 * p + hh
                    base = 64 * hh
                    qt_t, kt_t = qt_sb[p], kt_sb[p]
                    ytps = ps.tile([128, CH], F32, tag="ytps", bufs=1,
                                   name=f"ytps{hloc}_{c}")

                    def segs_of(ki):
                        q_off = max(0, 128 * ki - CH * c)
                        segs = []
                        if q_off < 512:
                            segs.append((q_off, 512))
                        segs.append((max(q_off, 512), CH))
                        return q_off, segs

                    def emit_s(ki):
                        q_off, segs = segs_of(ki)
                        stp = ps.tile([128, CH], F32, tag="stp", bufs=2,
                                      name=f"stp{hloc}_{c}_{ki}")
                        for (s0, s1) in segs:
                            nc.tensor.matmul(
                                stp[:, s0:s1],
                                lhsT=kt_t[base:base + 64,
                                          ki * 128:(ki + 1) * 128],
                                rhs=qt_t[base:base + 64,
                                         CH * c + s0:CH * c + s1],
                                start=True, stop=True,
                            )
                        pt = ptp.tile([128, CH], BF16, tag="pt",
                                      name=f"pt{hloc}_{c}_{ki}")
                        nc.scalar.activation(
                            out=pt[:, q_off:CH], in_=stp[:, q_off:CH],
                            func=AF.Exp, scale=SCALE,
                        )
                        if ki >= 8 * c:  # causal mask on diagonal block
                            nc.vector.tensor_mul(
                                pt[:, q_off:q_off + 128],
                                pt[:, q_off:q_off + 128], triu_sb,
                            )
                        return pt

                    b0_last = min(kmax - 1, 8 * c + 3)

                    def emit_av(ki, pt):
                        q_off, segs = segs_of(ki)
                        for (s0, s1) in segs:
                            last = b0_last if s0 < 512 else kmax - 1
                            nc.tensor.matmul(
                                ytps[:, s0:s1],
                                lhsT=v_sb[ki][:, 128 * hloc:128 * hloc + 128],
                                rhs=pt[:, s0:s1],
                                start=(ki == 0), stop=(ki == last),
                            )

                    # 1-deep software pipeline: S(ki+1) before AV(ki), with
                    # filler ops popped where PE would wait for exp(ki-1).
                    # c1 uses 2/ki so queued units (and their DVE copies)
                    # fully emit BEFORE this chunk's normalize chain hits
                    # the DVE FIFO — the tail out-projection depends on it.
                    nfill = 1
                    prev_pt = emit_s(0)
                    for ki in range(1, kmax):
                        pt = emit_s(ki)
                        fill(nfill)
                        emit_av(ki - 1, prev_pt)
                        prev_pt = pt
                    fill(nfill)
                    emit_av(kmax - 1, prev_pt)

                    # normalize: y * (1/den). den sits on psum rows 64:127.
                    # Engine lanes are partition-locked: every compute op
                    # stays partition-aligned; the 64->0 partition move is an
                    # SBUF->SBUF DMA (engine-free, HW-verified pattern).
                    # Drain ytps through TWO engines in parallel — DVE
                    # reciprocal of den (rows 64:128) and ACT Identity copy
                    # of y (rows 0:64) — so ytps frees in ~1.3us (< the next
                    # head-chunk's S(0)+S(1) PE time); the DMA shift and the
                    # all-SBUF bf16 multiply trail off the critical path.
                    rcp_sb = rcpp.tile([128, CH], BF16, tag="rcp",
                                       name=f"rcp{hloc}_{c}")
                    ytu_sb = ytup.tile([64, CH], BF16, tag="ytu",
                                       name=f"ytu{hloc}_{c}")
                    with nc.allow_low_precision(
                        reason="1/denominator in bf16: 0.4% rel on a "
                               "well-conditioned positive sum, budget 2e-2"
                    ):
                        nc.vector.reciprocal(
                            out=rcp_sb[64:128, :], in_=ytps[64:128, :]
                        )
                    nc.scalar.activation(
                        out=ytu_sb, in_=ytps[0:64, :],
                        func=AF.Identity, scale=1.0,
                    )
                    # split the shift+mul per column half so out-proj
                    # consumers unblock sooner
                    for (d0, d1) in ((0, 512), (512, CH)):
                        nc.sync.dma_start(
                            out=rcp_sb[0:64, d0:d1],
                            in_=rcp_sb[64:128, d0:d1],
                        )
                        nc.vector.tensor_mul(
                            yt_sb[p][base:base + 64, CH * c + d0:CH * c + d1],
                            ytu_sb[:, d0:d1], rcp_sb[0:64, d0:d1],
                        )

            def emit_out(qt_i, act_halves=(), split_dma=False):
                """Output projection for query tile qt_i + DMA to dram.

                act_halves: column halves whose psum->sbuf copy goes to the
                Activation engine — only safe once attention exp work there
                is done (ACT otherwise delays the exp stream).
                """
                drain_all()
                ot = outp.tile([128, C], BF16, tag="ot", name=f"ot{qt_i}")
                for s0 in (0, 512):
                    pso = ps.tile([128, 512], F32, tag="small", bufs=2,
                                  name=f"pso{qt_i}_{s0}")
                    for p in range(NPAIR):
                        nc.tensor.matmul(
                            pso,
                            lhsT=yt_sb[p][:, qt_i * 128:(qt_i + 1) * 128],
                            rhs=wp_sb[:, p, s0:s0 + 512],
                            start=(p == 0), stop=(p == NPAIR - 1),
                        )
                    if s0 in act_halves:
                        nc.scalar.activation(
                            out=ot[:, s0:s0 + 512], in_=pso,
                            func=AF.Copy, scale=1.0,
                        )
                    else:
                        nc.vector.tensor_copy(ot[:, s0:s0 + 512], pso)
                    if split_dma:  # final tiles: drain each half immediately
                        nc.sync.dma_start(
                            out=out.ap()[qt_i * 128:(qt_i + 1) * 128,
                                         s0:s0 + 512],
                            in_=ot[:, s0:s0 + 512],
                        )
                if not split_dma:
                    nc.sync.dma_start(
                        out=out.ap()[qt_i * 128:(qt_i + 1) * 128, :], in_=ot
                    )

            # ================= schedule =====================================
            for t in range(8):
                emit_v(t)
            for p in range(NPAIR):
                emit_qk(p, 0)
                # V tiles 8..15 become intra-attention fillers; pair 0 gets
                # none (its xt tail columns are still in flight on DMA).
                if p >= 1:
                    queue_v(6 + 2 * p)
                    queue_v(7 + 2 * p)
                emit_att(p, 0)
            queue_v(14)
            queue_v(15)
            while fill_q:  # V(14), V(15) + anything the slots didn't absorb
                fill(1)
            qk3_cnt = None
            for p in range(NPAIR):
                if p < NPAIR - 1:
                    emit_qk(p, 1)
                else:
                    # qk(3,ch1) was queued into att(2,c1); make sure every
                    # one of its ops is emitted before att(3,c1) reads qt/kt
                    while qk3_cnt["n"] > 0:
                        fill(1)
                # out-proj qtiles 0..7 (chunk-0 queries, ready since c0 pass)
                # become intra-attention fillers for the c1 pass; qk(3,ch1)
                # is queued (FIFO-first) into att(2,c1) instead of a block.
                if p == NPAIR - 2:
                    qk3_cnt = queue_qk(NPAIR - 1, 1)
                queue_out(2 * p)
                queue_out(2 * p + 1)
                emit_att(p, 1)
            while fill_q:
                fill(1)
            for qt_i in range(8, KT):
                # alternate whole-qtile copy engine so neither DVE nor ACT
                # serializes the tail
                halves = (0, 512) if qt_i % 2 else ()
                emit_out(qt_i, act_halves=halves, split_dma=True)

    nc.compile()
    _prog_cache[key] = nc
    return nc


def shard_inputs(x, W_qkv, b_qkv, W_proj, core):
    b, g = core // 2, core % 2
    cq = slice(512 * g, 512 * g + 512)
    ck = slice(1024 + 512 * g, 1024 + 512 * g + 512)
    cv = slice(2048 + 512 * g, 2048 + 512 * g + 512)
    return {
        "xt": np.ascontiguousarray(x[b].T).astype(NPBF16),
        "wq": np.ascontiguousarray(W_qkv[:, cq]).astype(NPBF16),
        "wk": np.ascontiguousarray(W_qkv[:, ck]).astype(NPBF16),
        "wv": np.ascontiguousarray(W_qkv[:, cv]).astype(NPBF16),
        "bqk_t": np.stack(
            [b_qkv[cq].reshape(4, 128)[p_] for p_ in range(4)]
            + [b_qkv[ck].reshape(4, 128)[p_] for p_ in range(4)], axis=1
        ).astype(np.float32).copy(),
        "bv": np.ascontiguousarray(b_qkv[cv]).reshape(1, 512).astype(NPBF16),
        "wp": np.ascontiguousarray(W_proj[512 * g:512 * g + 512, :]).astype(NPBF16),
    }


def kernel(x, W_qkv, b_qkv, W_proj, b_proj, **run_kwargs):
    x = np.asarray(x, np.float32)
    W_qkv = np.asarray(W_qkv, np.float32)
    b_qkv = np.asarray(b_qkv, np.float32)
    W_proj = np.asarray(W_proj, np.float32)
    b_proj = np.asarray(b_proj, np.float32)

    nc = build_program()
    in_maps = [
        shard_inputs(x, W_qkv, b_qkv, W_proj, core) for core in range(NCORES)
    ]
    from concourse.bass_utils import run_bass_kernel_spmd

    res = run_bass_kernel_spmd(nc, in_maps, core_ids=list(range(NCORES)), **run_kwargs)
    outs = [np.asarray(r["out"], np.float32) for r in res.results]
    full = np.stack([outs[2 * b_] + outs[2 * b_ + 1] + b_proj for b_ in range(B)])
    kernel.last_results = res
    return full


# revision 61
# speedup vs baseline: 1.0291x; 1.0291x over previous
"""Causal self-attention (B=4, T=2048, C=1024, H=16, D=64) on 8 TRN2 NeuronCores.

Sharding: core = 2*b + g  (b = batch 0..3, g = head-group 0..1; heads 8g..8g+7).
Each core computes, for its batch b and its 8 heads:
  qkv projection, causal softmax attention, and a PARTIAL output projection
  (its 512 rows of W_proj). Host sums the two partials per batch (+ b_proj).

Final design (301.7us baseline -> 262.6us; bf16 matmuls, f32 PSUM accum):
  - all matmul operands bf16: 1 cycle/row on PE at ANY free size (fp32r pays
    4x below 256 cols), halved DMA and SBUF footprint. bf16 noise ~3e-3 rel
    vs the 2e-2 budget (fp8 would blow it: ~5% q/k error -> ~0.2 abs).
  - softmax denominator for free: AV lhsT = [V_h | ones64] (128 cols), so
    PSUM rows 64:127 = sum_k P replicated across 64 partitions. Normalize =
    DVE reciprocal (partition-aligned 64:128) + ACT Identity copy of y,
    then SBUF->SBUF DMA shifts 1/den down to partitions 0:63 (engines are
    partition-locked; only DMA/PE can move data across partitions) + DVE
    multiply. Only ONE psum operand per vector op is legal.
  - consolidated DMAs: transfers run FIFO in descriptor order at ~350 B/ns
    with ~625ns serial descriptor gen per queue, so emission order IS the
    transfer schedule; front-load exactly what the V phase consumes first.
  - chunk-outer schedule with a filler queue: V(t0..7); {qk(p,ch0);
    att(p,c0) + V(t8..15) fillers} x4; {qk(p,ch1); att(p,c1) + out-proj
    qtile 0..7 fillers} x4; out-proj 8..15. The exp stream on ACT
    (0.83ns/col, ~152us) is the attention-phase co-bottleneck; popping one
    independent PE op between S(ki) and AV(ki-1) keeps PE fed through exp
    latency. Fillers and direct emitters share the "small" psum tag, so the
    queue must fully drain before any direct allocation (buffer rotation
    would corrupt an in-flight accumulation).
  - 1-deep software pipeline inside a head-chunk: emit S(ki+1) before AV(ki).
  - PE warmup: throwaway matmuls during the initial DMA wait ramp the PE
    p-state (1.54 -> 0.42 ns/col) before real work arrives.
  - PSUM banks (8): stp [128,1024] bufs=2 (4) + ytps [128,1024] (2) +
    small [128,512] bufs=2 (2).
  - psum->SBUF copy engines chosen to keep FIFOs clear: qk q-side DVE,
    k-side ACT Identity (per-partition bias AP); out-proj tail alternates
    DVE/ACT; V copies DVE; v-ones memsets per-tile on DVE (strided).
"""

import sys

try:
    import concourse  # noqa: F401
except ImportError:
    sys.path.insert(0, "/opt/trn_rl_repo")

import numpy as np
import ml_dtypes

import concourse.bacc as bacc
import concourse.mybir as mybir
import concourse.tile as tile

F32 = mybir.dt.float32
BF16 = mybir.dt.bfloat16
AF = mybir.ActivationFunctionType
ALU = mybir.AluOpType

B, T, C = 4, 2048, 1024
H, D = 16, 64
NCORES = 8
HL = 8          # heads per core (local)
NPAIR = 4       # head pairs per core
CH = 1024       # query chunk
NCH = T // CH   # 2
KT = T // 128   # 16 key tiles
CT = C // 128   # 8 contraction tiles over C
SCALE = 1.0 / 8.0  # 1/sqrt(D)

NPBF16 = ml_dtypes.bfloat16

_prog_cache = {}


def build_program(debug=False):
    key = debug
    if key in _prog_cache:
        return _prog_cache[key]

    nc = bacc.Bacc(None, target_bir_lowering=False, debug=debug)

    xt = nc.dram_tensor("xt", [C, T], BF16, kind="ExternalInput")
    wq = nc.dram_tensor("wq", [C, 512], BF16, kind="ExternalInput")
    wk = nc.dram_tensor("wk", [C, 512], BF16, kind="ExternalInput")
    wv = nc.dram_tensor("wv", [C, 512], BF16, kind="ExternalInput")
    bqk_t = nc.dram_tensor("bqk_t", [128, 8], F32, kind="ExternalInput")
    bv = nc.dram_tensor("bv", [1, 512], BF16, kind="ExternalInput")
    wp = nc.dram_tensor("wp", [512, C], BF16, kind="ExternalInput")
    out = nc.dram_tensor("out", [T, C], BF16, kind="ExternalOutput")

    with tile.TileContext(nc) as tc:
        with (
            tc.tile_pool(name="consts", bufs=1) as consts,
            tc.tile_pool(name="xtp", bufs=1) as xtp,
            tc.tile_pool(name="wvp", bufs=1) as wvp,
            tc.tile_pool(name="w8p", bufs=1) as w8p,
            tc.tile_pool(name="wpp", bufs=1) as wpp,
            tc.tile_pool(name="vp", bufs=1) as vp,
            tc.tile_pool(name="qkp", bufs=1) as qkp,
            tc.tile_pool(name="ytp", bufs=1) as ytp,
            tc.tile_pool(name="ptp", bufs=6) as ptp,
            tc.tile_pool(name="rcpp", bufs=2) as rcpp,
            tc.tile_pool(name="ytup", bufs=2) as ytup,
            tc.tile_pool(name="outp", bufs=3) as outp,
            tc.tile_pool(name="ps", bufs=1, space="PSUM") as ps,
        ):
            # ================= DMA staging (emission order = priority) ======
            # Transfers run FIFO in descriptor order on the shared DMA-engine
            # pool (~350 B/ns aggregate) and descriptor gen is ~625ns serial
            # per queue — so emission order IS the transfer schedule. One SP
            # chain, ordered by when the compute consumes each piece.
            vecs = consts.tile([128, 640], BF16, tag="vecs")
            bv_sb = vecs[32:33, 0:512]
            nc.sync.dma_start(out=bv_sb, in_=bv[:, :])
            xt_sb = xtp.tile([128, CT, T], BF16, tag="xt")
            xt_r = xt.ap().rearrange("(k p) t -> p k t", p=128)
            nc.sync.dma_start(out=xt_sb[:, :, 0:128], in_=xt_r[:, :, 0:128])
            wv_sb = wvp.tile([128, CT, 512], BF16, tag="wv")
            wv_r = wv.ap().rearrange("(k p) n -> p k n", p=128)
            for (k0_, k1_) in ((0, 1), (1, 4), (4, CT)):
                nc.sync.dma_start(
                    out=wv_sb[:, k0_:k1_, :], in_=wv_r[:, k0_:k1_, :]
                )
            for (c0_, c1_) in ((128, 256), (256, 512), (512, 1024)):
                nc.sync.dma_start(
                    out=xt_sb[:, :, c0_:c1_], in_=xt_r[:, :, c0_:c1_]
                )

            bqk_sb = consts.tile([128, 8], F32, tag="bqk")
            nc.sync.dma_start(out=bqk_sb, in_=bqk_t[:, :])

            # qk weights: w8[(p, side)] = [128, CT, 128] bf16 (one desc each)
            w8_sb = {}
            for p in range(NPAIR):
                for side, wsrc in ((0, wq), (1, wk)):
                    t_ = w8p.tile([128, CT, 128], BF16, tag=f"w8_{p}_{side}")
                    w_src = wsrc.ap().rearrange(
                        "(k pp) m -> pp k m", pp=128
                    )[:, :, p * 128:(p + 1) * 128]
                    nc.sync.dma_start(out=t_, in_=w_src)
                    w8_sb[(p, side)] = t_

            # wp: one descriptor, [128, NPAIR, 1024]
            wp_sb = wpp.tile([128, NPAIR, C], BF16, tag="wp")
            nc.sync.dma_start(
                out=wp_sb, in_=wp.ap().rearrange("(k p) n -> p k n", p=128)
            )
            # xt tail columns — only needed from V(t8..15) / qk ch1 onward
            for (c0_, c1_) in ((1024, 1536), (1536, 2048)):
                nc.sync.dma_start(
                    out=xt_sb[:, :, c0_:c1_], in_=xt_r[:, :, c0_:c1_]
                )

            # constants
            ones_f32 = consts.tile([128, 128], F32, tag="ones_f32")
            nc.vector.memset(ones_f32, 1.0)
            triu_f32 = consts.tile([128, 128], F32, tag="triu_f32")
            nc.gpsimd.memset(triu_f32, 1.0)
            nc.gpsimd.affine_select(
                out=triu_f32, in_=triu_f32,
                compare_op=ALU.is_ge,
                fill=0.0, base=0, pattern=[[1, 128]], channel_multiplier=-1,
            )
            triu_sb = consts.tile([128, 128], BF16, tag="triu")
            nc.vector.tensor_copy(triu_sb, triu_f32)
            # rank-1 v-bias vector ones: lhsT/rhs share base partition 32
            ones32_sb = vecs[32:33, 512:640]
            nc.vector.tensor_copy(ones32_sb, ones_f32[0:1, 0:128])

            # ================= persistent SBUF state ========================
            # v tiles: [128, 8 heads x (64 v-dims | 64 ones)]
            v_sb = []
            for t in range(KT):
                t_ = vp.tile([128, HL * 128], BF16, tag=f"v{t}", name=f"v{t}")
                v_sb.append(t_)

            def memset_v_ones(t):
                # ones columns only (strided): DVE, emitted per-tile right
                # before its V projection so 16 memsets don't jam the DVE
                # FIFO ahead of the V-phase psum copies. (gpsimd.memset on a
                # bf16 tile produced wrong bits on HW — keep this on DVE.)
                v_r = v_sb[t].rearrange("p (h x) -> p h x", h=HL)
                nc.vector.memset(v_r[:, :, 64:128], 1.0)
            qt_sb = [qkp.tile([128, T], BF16, tag=f"qt{p}", name=f"qt{p}")
                     for p in range(NPAIR)]
            kt_sb = [qkp.tile([128, T], BF16, tag=f"kt{p}", name=f"kt{p}")
                     for p in range(NPAIR)]
            yt_sb = [ytp.tile([128, T], BF16, tag=f"yt{p}", name=f"yt{p}")
                     for p in range(NPAIR)]

            # ---- PE warmup: the tensor engine ramps (1.54 -> 0.83 -> 0.42
            # ns/col over ~3us of continuous work). Burn the initial DMA wait
            # on throwaway matmuls over the freshly-memset v tile so the real
            # V projection starts at full clock.
            warm = ps.tile([128, CH], F32, tag="stp", bufs=2, name="warm")
            for i_ in range(7):
                s0 = 512 * (i_ % 2)
                nc.tensor.matmul(
                    warm[:, s0:s0 + 512],
                    lhsT=v_sb[0][:, 0:128], rhs=v_sb[0][:, 0:512],
                    start=True, stop=True,
                )

            # ================= filler queue =================================
            # The exp stream on ACT runs ~0.2us/ki slower than PE's S+AV, so
            # stalls accumulate inside a head-chunk. Queue independent PE work
            # (V tiles, out-proj qtiles) as single-matmul callables and pop
            # one between S(ki) and AV(ki-1) — PE chews filler exactly where
            # it would otherwise wait for exp(ki-1).
            from collections import deque
            fill_q = deque()

            def fill(n=1):
                for _ in range(n):
                    if not fill_q:
                        return
                    fill_q.popleft()()

            def drain_all():
                """MUST run before any direct 'small'/'ot' tile allocation:
                a queued unit left half-emitted would have its rotating psum
                buffer stolen mid-accumulation (silent corruption)."""
                while fill_q:
                    fill(1)

            def queue_v(t):
                """Enqueue V projection for key-tile t as per-op callables."""
                st = {}

                def mk_mm(k):
                    def f():
                        if k == 0:
                            st["pv"] = ps.tile([128, 512], F32, tag="small",
                                               bufs=2, name=f"pv{t}")
                        nc.tensor.matmul(
                            st["pv"],
                            lhsT=xt_sb[:, k, t * 128:(t + 1) * 128],
                            rhs=wv_sb[:, k, :], start=(k == 0), stop=False,
                        )
                    return f

                def bias():
                    nc.tensor.matmul(
                        st["pv"], lhsT=ones32_sb, rhs=bv_sb,
                        start=False, stop=True,
                    )

                def copy():
                    v_r = v_sb[t].rearrange("p (h x) -> p h x", h=HL)
                    pv_r = st["pv"].rearrange("p (h d) -> p h d", h=HL)
                    nc.vector.tensor_copy(v_r[:, :, 0:64], pv_r)

                fill_q.append(lambda: memset_v_ones(t))
                for k in range(CT):
                    fill_q.append(mk_mm(k))
                fill_q.append(bias)
                fill_q.append(copy)

            def queue_qk(p, ch):
                """Enqueue qk projection (filler variant: copies on DVE only,
                so no exp-stream interference when popped mid-attention).
                Returns a counter dict; drain until counter hits 0 before
                emitting anything that reads qt/kt of this pair+chunk."""
                st = {}
                cnt = {"n": 0}

                def wrap(f):
                    cnt["n"] += 1

                    def g():
                        f()
                        cnt["n"] -= 1
                    return g

                def mk_mm(side, s0, k):
                    def f():
                        if k == 0:
                            st[(side, s0)] = ps.tile(
                                [128, 512], F32, tag="small", bufs=2,
                                name=f"pq{p}_{side}_{ch}_{s0}")
                        nc.tensor.matmul(
                            st[(side, s0)],
                            lhsT=w8_sb[(p, side)][:, k, :],
                            rhs=xt_sb[:, k, ch * CH + s0:ch * CH + s0 + 512],
                            start=(k == 0), stop=(k == CT - 1),
                        )
                    return f

                def mk_copy(side, s0):
                    dst = qt_sb[p] if side == 0 else kt_sb[p]
                    bcol = bqk_sb[:, 4 * side + p:4 * side + p + 1]

                    def f():
                        nc.vector.tensor_scalar(
                            out=dst[:, ch * CH + s0:ch * CH + s0 + 512],
                            in0=st[(side, s0)], scalar1=bcol, scalar2=None,
                            op0=ALU.add,
                        )
                    return f

                for side in (0, 1):
                    for s0 in (0, 512):
                        for k in range(CT):
                            fill_q.append(wrap(mk_mm(side, s0, k)))
                        fill_q.append(wrap(mk_copy(side, s0)))
                return cnt

            def queue_out(qt_i):
                """Enqueue out-projection for query tile qt_i (DVE copies)."""
                st = {}

                def mk_mm(s0, p):
                    def f():
                        if p == 0:
                            st[s0] = ps.tile([128, 512], F32, tag="small",
                                             bufs=2, name=f"pso{qt_i}_{s0}")
                        nc.tensor.matmul(
                            st[s0],
                            lhsT=yt_sb[p][:, qt_i * 128:(qt_i + 1) * 128],
                            rhs=wp_sb[:, p, s0:s0 + 512],
                            start=(p == 0), stop=(p == NPAIR - 1),
                        )
                    return f

                def mk_copy(s0):
                    def f():
                        if "ot" not in st:
                            st["ot"] = outp.tile([128, C], BF16, tag="ot",
                                                 name=f"ot{qt_i}")
                        nc.vector.tensor_copy(
                            st["ot"][:, s0:s0 + 512], st[s0]
                        )
                    return f

                def dma():
                    nc.sync.dma_start(
                        out=out.ap()[qt_i * 128:(qt_i + 1) * 128, :],
                        in_=st["ot"],
                    )

                for s0 in (0, 512):
                    for p in range(NPAIR):
                        fill_q.append(mk_mm(s0, p))
                    fill_q.append(mk_copy(s0))
                fill_q.append(dma)

            # ================= phase emitters ===============================
            def emit_v(t):
                """V projection for key-tile t -> v_sb[t]."""
                memset_v_ones(t)
                pv = ps.tile([128, 512], F32, tag="small", bufs=2,
                             name=f"pv{t}")
                for k in range(CT):
                    nc.tensor.matmul(
                        pv,
                        lhsT=xt_sb[:, k, t * 128:(t + 1) * 128],
                        rhs=wv_sb[:, k, :], start=(k == 0), stop=False,
                    )
                nc.tensor.matmul(
                    pv, lhsT=ones32_sb, rhs=bv_sb,
                    start=False, stop=True,
                )
                v_r = v_sb[t].rearrange("p (h x) -> p h x", h=HL)
                pv_r = pv.rearrange("p (h d) -> p h d", h=HL)
                nc.vector.tensor_copy(v_r[:, :, 0:64], pv_r)

            def emit_qk(p, ch):
                """qk projection for pair p, T-chunk ch -> qt/kt cols."""
                drain_all()
                for side, dst in ((0, qt_sb[p]), (1, kt_sb[p])):
                    w8 = w8_sb[(p, side)]
                    bcol = bqk_sb[:, 4 * side + p:4 * side + p + 1]
                    for s0 in (0, 512):
                        pq = ps.tile([128, 512], F32, tag="small", bufs=2,
                                     name=f"pq{p}_{side}_{ch}_{s0}")
                        for k in range(CT):
                            nc.tensor.matmul(
                                pq,
                                lhsT=w8[:, k, :],
                                rhs=xt_sb[:, k,
                                          ch * CH + s0:ch * CH + s0 + 512],
                                start=(k == 0), stop=(k == CT - 1),
                            )
                        # q-side copies on DVE, k-side on ACT (GPSIMD cannot
                        # read PSUM): the first S matmul needs BOTH qt and kt
                        # — parallel engines halve that critical-path latency.
                        if side == 0:
                            nc.vector.tensor_scalar(
                                out=dst[:, ch * CH + s0:ch * CH + s0 + 512],
                                in0=pq, scalar1=bcol, scalar2=None,
                                op0=ALU.add,
                            )
                        else:
                            nc.scalar.activation(
                                out=dst[:, ch * CH + s0:ch * CH + s0 + 512],
                                in_=pq, func=AF.Identity, bias=bcol, scale=1.0,
                            )

            def emit_att(p, c):
                """Attention for pair p's two heads over query chunk c."""
                kmax = 8 * (c + 1)
                for hh in range(2):
                    hloc = 2 * p + hh
                    base = 64 * hh
                    qt_t, kt_t = qt_sb[p], kt_sb[p]
                    ytps = ps.tile([128, CH], F32, tag="ytps", bufs=1,
                                   name=f"ytps{hloc}_{c}")

                    def segs_of(ki):
                        q_off = max(0, 128 * ki - CH * c)
                        segs = []
                        if q_off < 512:
                            segs.append((q_off, 512))
                        segs.append((max(q_off, 512), CH))
                        return q_off, segs

                    def emit_s(ki):
                        q_off, segs = segs_of(ki)
                        stp = ps.tile([128, CH], F32, tag="stp", bufs=2,
                                      name=f"stp{hloc}_{c}_{ki}")
                        for (s0, s1) in segs:
                            nc.tensor.matmul(
                                stp[:, s0:s1],
                                lhsT=kt_t[base:base + 64,
                                          ki * 128:(ki + 1) * 128],
                                rhs=qt_t[base:base + 64,
                                         CH * c + s0:CH * c + s1],
                                start=True, stop=True,
                            )
                        pt = ptp.tile([128, CH], BF16, tag="pt",
                                      name=f"pt{hloc}_{c}_{ki}")
                        nc.scalar.activation(
                            out=pt[:, q_off:CH], in_=stp[:, q_off:CH],
                            func=AF.Exp, scale=SCALE,
                        )
                        if ki >= 8 * c:  # causal mask on diagonal block
                            nc.vector.tensor_mul(
                                pt[:, q_off:q_off + 128],
                                pt[:, q_off:q_off + 128], triu_sb,
                            )
                        return pt

                    b0_last = min(kmax - 1, 8 * c + 3)

                    def emit_av(ki, pt):
                        q_off, segs = segs_of(ki)
                        for (s0, s1) in segs:
                            last = b0_last if s0 < 512 else kmax - 1
                            nc.tensor.matmul(
                                ytps[:, s0:s1],
                                lhsT=v_sb[ki][:, 128 * hloc:128 * hloc + 128],
                                rhs=pt[:, s0:s1],
                                start=(ki == 0), stop=(ki == last),
                            )

                    # 1-deep software pipeline: S(ki+1) before AV(ki), with
                    # filler ops popped where PE would wait for exp(ki-1).
                    # c1 uses 2/ki so queued units (and their DVE copies)
                    # fully emit BEFORE this chunk's normalize chain hits
                    # the DVE FIFO — the tail out-projection depends on it.
                    nfill = 1
                    prev_pt = emit_s(0)
                    for ki in range(1, kmax):
                        pt = emit_s(ki)
                        fill(nfill)
                        emit_av(ki - 1, prev_pt)
                        prev_pt = pt
                    fill(nfill)
                    emit_av(kmax - 1, prev_pt)

                    # normalize: y * (1/den). den sits on psum rows 64:127.
                    # Engine lanes are partition-locked: every compute op
                    # stays partition-aligned; the 64->0 partition move is an
                    # SBUF->SBUF DMA (engine-free, HW-verified pattern).
                    # Drain ytps through TWO engines in parallel — DVE
                    # reciprocal of den (rows 64:128) and ACT Identity copy
                    # of y (rows 0:64) — so ytps frees in ~1.3us (< the next
                    # head-chunk's S(0)+S(1) PE time); the DMA shift and the
                    # all-SBUF bf16 multiply trail off the critical path.
                    # DVE ops map lanes RELATIVELY within the partition
                    # range (the fp32r baseline did a cross-partition
                    # reciprocal on HW), so read den from rows 64:128 and
                    # write 1/den to rows 0:64 directly — no DMA shift, no
                    # ACT staging. Per column half so consumers (next
                    # head-chunk / out-proj) unblock sooner.
                    rcp_sb = rcpp.tile([64, CH], BF16, tag="rcp",
                                       name=f"rcp{hloc}_{c}")
                    for (d0, d1) in ((0, 512), (512, CH)):
                        with nc.allow_low_precision(
                            reason="1/denominator in bf16: 0.4% rel on a "
                                   "well-conditioned positive sum, budget 2e-2"
                        ):
                            nc.vector.reciprocal(
                                out=rcp_sb[:, d0:d1],
                                in_=ytps[64:128, d0:d1],
                            )
                        nc.vector.tensor_mul(
                            yt_sb[p][base:base + 64, CH * c + d0:CH * c + d1],
                            ytps[0:64, d0:d1], rcp_sb[:, d0:d1],
                        )

            def emit_out(qt_i, act_halves=(), split_dma=False):
                """Output projection for query tile qt_i + DMA to dram.

                act_halves: column halves whose psum->sbuf copy goes to the
                Activation engine — only safe once attention exp work there
                is done (ACT otherwise delays the exp stream).
                """
                drain_all()
                ot = outp.tile([128, C], BF16, tag="ot", name=f"ot{qt_i}")
                for s0 in (0, 512):
                    pso = ps.tile([128, 512], F32, tag="small", bufs=2,
                                  name=f"pso{qt_i}_{s0}")
                    for p in range(NPAIR):
                        nc.tensor.matmul(
                            pso,
                            lhsT=yt_sb[p][:, qt_i * 128:(qt_i + 1) * 128],
                            rhs=wp_sb[:, p, s0:s0 + 512],
                            start=(p == 0), stop=(p == NPAIR - 1),
                        )
                    if s0 in act_halves:
                        nc.scalar.activation(
                            out=ot[:, s0:s0 + 512], in_=pso,
                            func=AF.Copy, scale=1.0,
                        )
                    else:
                        nc.vector.tensor_copy(ot[:, s0:s0 + 512], pso)
                    if split_dma:  # final tiles: drain each half immediately
                        nc.sync.dma_start(
                            out=out.ap()[qt_i * 128:(qt_i + 1) * 128,
                                         s0:s0 + 512],
                            in_=ot[:, s0:s0 + 512],
                        )
                if not split_dma:
                    nc.sync.dma_start(
                        out=out.ap()[qt_i * 128:(qt_i + 1) * 128, :], in_=ot
                    )

            # ================= schedule =====================================
            for t in range(8):
                emit_v(t)
            for p in range(NPAIR):
                emit_qk(p, 0)
                # V tiles 8..15 become intra-attention fillers; pair 0 gets
                # none (its xt tail columns are still in flight on DMA).
                if p >= 1:
                    queue_v(6 + 2 * p)
                    queue_v(7 + 2 * p)
                emit_att(p, 0)
            queue_v(14)
            queue_v(15)
            while fill_q:  # V(14), V(15) + anything the slots didn't absorb
                fill(1)
            qk3_cnt = None
            for p in range(NPAIR):
                if p < NPAIR - 1:
                    emit_qk(p, 1)
                else:
                    # qk(3,ch1) was queued into att(2,c1); make sure every
                    # one of its ops is emitted before att(3,c1) reads qt/kt
                    while qk3_cnt["n"] > 0:
                        fill(1)
                # out-proj qtiles 0..7 (chunk-0 queries, ready since c0 pass)
                # become intra-attention fillers for the c1 pass; qk(3,ch1)
                # is queued (FIFO-first) into att(2,c1) instead of a block.
                if p == NPAIR - 2:
                    qk3_cnt = queue_qk(NPAIR - 1, 1)
                queue_out(2 * p)
                queue_out(2 * p + 1)
                emit_att(p, 1)
            while fill_q:
                fill(1)
            for qt_i in range(8, KT):
                # alternate whole-qtile copy engine so neither DVE nor ACT
                # serializes the tail
                halves = (0, 512) if qt_i % 2 else ()
                emit_out(qt_i, act_halves=halves, split_dma=True)

    nc.compile()
    _prog_cache[key] = nc
    return nc


def shard_inputs(x, W_qkv, b_qkv, W_proj, core):
    b, g = core // 2, core % 2
    cq = slice(512 * g, 512 * g + 512)
    ck = slice(1024 + 512 * g, 1024 + 512 * g + 512)
    cv = slice(2048 + 512 * g, 2048 + 512 * g + 512)
    return {
        "xt": np.ascontiguousarray(x[b].T).astype(NPBF16),
        "wq": np.ascontiguousarray(W_qkv[:, cq]).astype(NPBF16),
        "wk": np.ascontiguousarray(W_qkv[:, ck]).astype(NPBF16),
        "wv": np.ascontiguousarray(W_qkv[:, cv]).astype(NPBF16),
        "bqk_t": np.stack(
            [b_qkv[cq].reshape(4, 128)[p_] for p_ in range(4)]
            + [b_qkv[ck].reshape(4, 128)[p_] for p_ in range(4)], axis=1
        ).astype(np.float32).copy(),
        "bv": np.ascontiguousarray(b_qkv[cv]).reshape(1, 512).astype(NPBF16),
        "wp": np.ascontiguousarray(W_proj[512 * g:512 * g + 512, :]).astype(NPBF16),
    }


def kernel(x, W_qkv, b_qkv, W_proj, b_proj, **run_kwargs):
    x = np.asarray(x, np.float32)
    W_qkv = np.asarray(W_qkv, np.float32)
    b_qkv = np.asarray(b_qkv, np.float32)
    W_proj = np.asarray(W_proj, np.float32)
    b_proj = np.asarray(b_proj, np.float32)

    nc = build_program()
    in_maps = [
        shard_inputs(x, W_qkv, b_qkv, W_proj, core) for core in range(NCORES)
    ]
    from concourse.bass_utils import run_bass_kernel_spmd

    res = run_bass_kernel_spmd(nc, in_maps, core_ids=list(range(NCORES)), **run_kwargs)
    outs = [np.asarray(r["out"], np.float32) for r in res.results]
    full = np.stack([outs[2 * b_] + outs[2 * b_ + 1] + b_proj for b_ in range(B)])
    kernel.last_results = res
    return full


# revision 62
# speedup vs baseline: 1.0491x; 1.0195x over previous
"""Causal self-attention (B=4, T=2048, C=1024, H=16, D=64) on 8 TRN2 NeuronCores.

Sharding: core = 2*b + g  (b = batch 0..3, g = head-group 0..1; heads 8g..8g+7).
Each core computes, for its batch b and its 8 heads:
  qkv projection, causal softmax attention, and a PARTIAL output projection
  (its 512 rows of W_proj). Host sums the two partials per batch (+ b_proj).

Final design (301.7us baseline -> 262.6us; bf16 matmuls, f32 PSUM accum):
  - all matmul operands bf16: 1 cycle/row on PE at ANY free size (fp32r pays
    4x below 256 cols), halved DMA and SBUF footprint. bf16 noise ~3e-3 rel
    vs the 2e-2 budget (fp8 would blow it: ~5% q/k error -> ~0.2 abs).
  - softmax denominator for free: AV lhsT = [V_h | ones64] (128 cols), so
    PSUM rows 64:127 = sum_k P replicated across 64 partitions. Normalize =
    DVE reciprocal (partition-aligned 64:128) + ACT Identity copy of y,
    then SBUF->SBUF DMA shifts 1/den down to partitions 0:63 (engines are
    partition-locked; only DMA/PE can move data across partitions) + DVE
    multiply. Only ONE psum operand per vector op is legal.
  - consolidated DMAs: transfers run FIFO in descriptor order at ~350 B/ns
    with ~625ns serial descriptor gen per queue, so emission order IS the
    transfer schedule; front-load exactly what the V phase consumes first.
  - chunk-outer schedule with a filler queue: V(t0..7); {qk(p,ch0);
    att(p,c0) + V(t8..15) fillers} x4; {qk(p,ch1); att(p,c1) + out-proj
    qtile 0..7 fillers} x4; out-proj 8..15. The exp stream on ACT
    (0.83ns/col, ~152us) is the attention-phase co-bottleneck; popping one
    independent PE op between S(ki) and AV(ki-1) keeps PE fed through exp
    latency. Fillers and direct emitters share the "small" psum tag, so the
    queue must fully drain before any direct allocation (buffer rotation
    would corrupt an in-flight accumulation).
  - 1-deep software pipeline inside a head-chunk: emit S(ki+1) before AV(ki).
  - PE warmup: throwaway matmuls during the initial DMA wait ramp the PE
    p-state (1.54 -> 0.42 ns/col) before real work arrives.
  - PSUM banks (8): stp [128,1024] bufs=2 (4) + ytps [128,1024] (2) +
    small [128,512] bufs=2 (2).
  - psum->SBUF copy engines chosen to keep FIFOs clear: qk q-side DVE,
    k-side ACT Identity (per-partition bias AP); out-proj tail alternates
    DVE/ACT; V copies DVE; v-ones memsets per-tile on DVE (strided).
"""

import sys

try:
    import concourse  # noqa: F401
except ImportError:
    sys.path.insert(0, "/opt/trn_rl_repo")

import numpy as np
import ml_dtypes

import concourse.bacc as bacc
import concourse.mybir as mybir
import concourse.tile as tile

F32 = mybir.dt.float32
BF16 = mybir.dt.bfloat16
AF = mybir.ActivationFunctionType
ALU = mybir.AluOpType

B, T, C = 4, 2048, 1024
H, D = 16, 64
NCORES = 8
HL = 8          # heads per core (local)
NPAIR = 4       # head pairs per core
CH = 1024       # query chunk
NCH = T // CH   # 2
KT = T // 128   # 16 key tiles
CT = C // 128   # 8 contraction tiles over C
SCALE = 1.0 / 8.0  # 1/sqrt(D)

NPBF16 = ml_dtypes.bfloat16

_prog_cache = {}


def build_program(debug=False):
    key = debug
    if key in _prog_cache:
        return _prog_cache[key]

    nc = bacc.Bacc(None, target_bir_lowering=False, debug=debug)

    xt = nc.dram_tensor("xt", [C, T], BF16, kind="ExternalInput")
    wq = nc.dram_tensor("wq", [C, 512], BF16, kind="ExternalInput")
    wk = nc.dram_tensor("wk", [C, 512], BF16, kind="ExternalInput")
    wv = nc.dram_tensor("wv", [C, 512], BF16, kind="ExternalInput")
    bqk_t = nc.dram_tensor("bqk_t", [128, 8], F32, kind="ExternalInput")
    bv = nc.dram_tensor("bv", [1, 512], BF16, kind="ExternalInput")
    wp = nc.dram_tensor("wp", [512, C], BF16, kind="ExternalInput")
    out = nc.dram_tensor("out", [T, C], BF16, kind="ExternalOutput")

    with tile.TileContext(nc) as tc:
        with (
            tc.tile_pool(name="consts", bufs=1) as consts,
            tc.tile_pool(name="xtp", bufs=1) as xtp,
            tc.tile_pool(name="wvp", bufs=1) as wvp,
            tc.tile_pool(name="w8p", bufs=1) as w8p,
            tc.tile_pool(name="wpp", bufs=1) as wpp,
            tc.tile_pool(name="vp", bufs=1) as vp,
            tc.tile_pool(name="qkp", bufs=1) as qkp,
            tc.tile_pool(name="ytp", bufs=1) as ytp,
            tc.tile_pool(name="ptp", bufs=6) as ptp,
            tc.tile_pool(name="rcpp", bufs=2) as rcpp,
            tc.tile_pool(name="ytup", bufs=2) as ytup,
            tc.tile_pool(name="outp", bufs=3) as outp,
            tc.tile_pool(name="ps", bufs=1, space="PSUM") as ps,
        ):
            # ================= DMA staging (emission order = priority) ======
            # Transfers run FIFO in descriptor order on the shared DMA-engine
            # pool (~350 B/ns aggregate) and descriptor gen is ~625ns serial
            # per queue — so emission order IS the transfer schedule. One SP
            # chain, ordered by when the compute consumes each piece.
            vecs = consts.tile([128, 640], BF16, tag="vecs")
            bv_sb = vecs[32:33, 0:512]
            nc.sync.dma_start(out=bv_sb, in_=bv[:, :])
            wv_sb = wvp.tile([128, CT, 512], BF16, tag="wv")
            wv_r = wv.ap().rearrange("(k p) n -> p k n", p=128)
            nc.sync.dma_start(out=wv_sb[:, 0:1, :], in_=wv_r[:, 0:1, :])
            xt_sb = xtp.tile([128, CT, T], BF16, tag="xt")
            xt_r = xt.ap().rearrange("(k p) t -> p k t", p=128)
            nc.sync.dma_start(out=xt_sb[:, :, 0:128], in_=xt_r[:, :, 0:128])
            for (k0_, k1_) in ((1, 4), (4, CT)):
                nc.sync.dma_start(
                    out=wv_sb[:, k0_:k1_, :], in_=wv_r[:, k0_:k1_, :]
                )
            for (c0_, c1_) in ((128, 256), (256, 512), (512, 1024)):
                nc.sync.dma_start(
                    out=xt_sb[:, :, c0_:c1_], in_=xt_r[:, :, c0_:c1_]
                )

            bqk_sb = consts.tile([128, 8], F32, tag="bqk")
            nc.sync.dma_start(out=bqk_sb, in_=bqk_t[:, :])

            # qk weights: w8[(p, side)] = [128, CT, 128] bf16 (one desc each)
            w8_sb = {}
            for p in range(NPAIR):
                for side, wsrc in ((0, wq), (1, wk)):
                    t_ = w8p.tile([128, CT, 128], BF16, tag=f"w8_{p}_{side}")
                    w_src = wsrc.ap().rearrange(
                        "(k pp) m -> pp k m", pp=128
                    )[:, :, p * 128:(p + 1) * 128]
                    nc.sync.dma_start(out=t_, in_=w_src)
                    w8_sb[(p, side)] = t_

            # wp: one descriptor, [128, NPAIR, 1024]
            wp_sb = wpp.tile([128, NPAIR, C], BF16, tag="wp")
            nc.sync.dma_start(
                out=wp_sb, in_=wp.ap().rearrange("(k p) n -> p k n", p=128)
            )
            # xt tail columns — only needed from V(t8..15) / qk ch1 onward
            for (c0_, c1_) in ((1024, 1536), (1536, 2048)):
                nc.sync.dma_start(
                    out=xt_sb[:, :, c0_:c1_], in_=xt_r[:, :, c0_:c1_]
                )

            # constants
            ones_f32 = consts.tile([128, 128], F32, tag="ones_f32")
            nc.vector.memset(ones_f32, 1.0)
            triu_f32 = consts.tile([128, 128], F32, tag="triu_f32")
            nc.gpsimd.memset(triu_f32, 1.0)
            nc.gpsimd.affine_select(
                out=triu_f32, in_=triu_f32,
                compare_op=ALU.is_ge,
                fill=0.0, base=0, pattern=[[1, 128]], channel_multiplier=-1,
            )
            triu_sb = consts.tile([128, 128], BF16, tag="triu")
            nc.vector.tensor_copy(triu_sb, triu_f32)
            # rank-1 v-bias vector ones: lhsT/rhs share base partition 32
            ones32_sb = vecs[32:33, 512:640]
            nc.vector.tensor_copy(ones32_sb, ones_f32[0:1, 0:128])

            # ================= persistent SBUF state ========================
            # v tiles: [128, 8 heads x (64 v-dims | 64 ones)]
            v_sb = []
            for t in range(KT):
                t_ = vp.tile([128, HL * 128], BF16, tag=f"v{t}", name=f"v{t}")
                v_sb.append(t_)

            def memset_v_ones(t):
                # ones columns only (strided): DVE, emitted per-tile right
                # before its V projection so 16 memsets don't jam the DVE
                # FIFO ahead of the V-phase psum copies. (gpsimd.memset on a
                # bf16 tile produced wrong bits on HW — keep this on DVE.)
                v_r = v_sb[t].rearrange("p (h x) -> p h x", h=HL)
                nc.vector.memset(v_r[:, :, 64:128], 1.0)
            qt_sb = [qkp.tile([128, T], BF16, tag=f"qt{p}", name=f"qt{p}")
                     for p in range(NPAIR)]
            kt_sb = [qkp.tile([128, T], BF16, tag=f"kt{p}", name=f"kt{p}")
                     for p in range(NPAIR)]
            yt_sb = [ytp.tile([128, T], BF16, tag=f"yt{p}", name=f"yt{p}")
                     for p in range(NPAIR)]

            # ---- PE warmup: the tensor engine ramps (1.54 -> 0.83 -> 0.42
            # ns/col over ~3us of continuous work). Burn the initial DMA wait
            # on throwaway matmuls over the freshly-memset v tile so the real
            # V projection starts at full clock.
            warm = ps.tile([128, CH], F32, tag="stp", bufs=2, name="warm")
            for i_ in range(8):
                s0 = 512 * (i_ % 2)
                nc.tensor.matmul(
                    warm[:, s0:s0 + 512],
                    lhsT=v_sb[0][:, 0:128], rhs=v_sb[0][:, 0:512],
                    start=True, stop=True,
                )

            # ================= filler queue =================================
            # The exp stream on ACT runs ~0.2us/ki slower than PE's S+AV, so
            # stalls accumulate inside a head-chunk. Queue independent PE work
            # (V tiles, out-proj qtiles) as single-matmul callables and pop
            # one between S(ki) and AV(ki-1) — PE chews filler exactly where
            # it would otherwise wait for exp(ki-1).
            from collections import deque
            fill_q = deque()

            def fill(n=1):
                for _ in range(n):
                    if not fill_q:
                        return
                    fill_q.popleft()()

            def drain_all():
                """MUST run before any direct 'small'/'ot' tile allocation:
                a queued unit left half-emitted would have its rotating psum
                buffer stolen mid-accumulation (silent corruption)."""
                while fill_q:
                    fill(1)

            def queue_v(t):
                """Enqueue V projection for key-tile t as per-op callables."""
                st = {}

                def mk_mm(k):
                    def f():
                        if k == 0:
                            st["pv"] = ps.tile([128, 512], F32, tag="small",
                                               bufs=2, name=f"pv{t}")
                        nc.tensor.matmul(
                            st["pv"],
                            lhsT=xt_sb[:, k, t * 128:(t + 1) * 128],
                            rhs=wv_sb[:, k, :], start=(k == 0), stop=False,
                        )
                    return f

                def bias():
                    nc.tensor.matmul(
                        st["pv"], lhsT=ones32_sb, rhs=bv_sb,
                        start=False, stop=True,
                    )

                def copy():
                    v_r = v_sb[t].rearrange("p (h x) -> p h x", h=HL)
                    pv_r = st["pv"].rearrange("p (h d) -> p h d", h=HL)
                    nc.vector.tensor_copy(v_r[:, :, 0:64], pv_r)

                fill_q.append(lambda: memset_v_ones(t))
                for k in range(CT):
                    fill_q.append(mk_mm(k))
                fill_q.append(bias)
                fill_q.append(copy)

            def queue_qk(p, ch):
                """Enqueue qk projection (filler variant: copies on DVE only,
                so no exp-stream interference when popped mid-attention).
                Returns a counter dict; drain until counter hits 0 before
                emitting anything that reads qt/kt of this pair+chunk."""
                st = {}
                cnt = {"n": 0}

                def wrap(f):
                    cnt["n"] += 1

                    def g():
                        f()
                        cnt["n"] -= 1
                    return g

                def mk_mm(side, s0, k):
                    def f():
                        if k == 0:
                            st[(side, s0)] = ps.tile(
                                [128, 512], F32, tag="small", bufs=2,
                                name=f"pq{p}_{side}_{ch}_{s0}")
                        nc.tensor.matmul(
                            st[(side, s0)],
                            lhsT=w8_sb[(p, side)][:, k, :],
                            rhs=xt_sb[:, k, ch * CH + s0:ch * CH + s0 + 512],
                            start=(k == 0), stop=(k == CT - 1),
                        )
                    return f

                def mk_copy(side, s0):
                    dst = qt_sb[p] if side == 0 else kt_sb[p]
                    bcol = bqk_sb[:, 4 * side + p:4 * side + p + 1]

                    def f():
                        nc.vector.tensor_scalar(
                            out=dst[:, ch * CH + s0:ch * CH + s0 + 512],
                            in0=st[(side, s0)], scalar1=bcol, scalar2=None,
                            op0=ALU.add,
                        )
                    return f

                for side in (0, 1):
                    for s0 in (0, 512):
                        for k in range(CT):
                            fill_q.append(wrap(mk_mm(side, s0, k)))
                        fill_q.append(wrap(mk_copy(side, s0)))
                return cnt

            def queue_out(qt_i):
                """Enqueue out-projection for query tile qt_i (DVE copies)."""
                st = {}

                def mk_mm(s0, p):
                    def f():
                        if p == 0:
                            st[s0] = ps.tile([128, 512], F32, tag="small",
                                             bufs=2, name=f"pso{qt_i}_{s0}")
                        nc.tensor.matmul(
                            st[s0],
                            lhsT=yt_sb[p][:, qt_i * 128:(qt_i + 1) * 128],
                            rhs=wp_sb[:, p, s0:s0 + 512],
                            start=(p == 0), stop=(p == NPAIR - 1),
                        )
                    return f

                def mk_copy(s0):
                    def f():
                        if "ot" not in st:
                            st["ot"] = outp.tile([128, C], BF16, tag="ot",
                                                 name=f"ot{qt_i}")
                        nc.vector.tensor_copy(
                            st["ot"][:, s0:s0 + 512], st[s0]
                        )
                    return f

                def dma():
                    nc.sync.dma_start(
                        out=out.ap()[qt_i * 128:(qt_i + 1) * 128, :],
                        in_=st["ot"],
                    )

                for s0 in (0, 512):
                    for p in range(NPAIR):
                        fill_q.append(mk_mm(s0, p))
                    fill_q.append(mk_copy(s0))
                fill_q.append(dma)

            # ================= phase emitters ===============================
            def emit_v(t):
                """V projection for key-tile t -> v_sb[t]."""
                memset_v_ones(t)
                pv = ps.tile([128, 512], F32, tag="small", bufs=2,
                             name=f"pv{t}")
                for k in range(CT):
                    nc.tensor.matmul(
                        pv,
                        lhsT=xt_sb[:, k, t * 128:(t + 1) * 128],
                        rhs=wv_sb[:, k, :], start=(k == 0), stop=False,
                    )
                nc.tensor.matmul(
                    pv, lhsT=ones32_sb, rhs=bv_sb,
                    start=False, stop=True,
                )
                v_r = v_sb[t].rearrange("p (h x) -> p h x", h=HL)
                pv_r = pv.rearrange("p (h d) -> p h d", h=HL)
                nc.vector.tensor_copy(v_r[:, :, 0:64], pv_r)

            def emit_qk(p, ch):
                """qk projection for pair p, T-chunk ch -> qt/kt cols."""
                drain_all()
                for side, dst in ((0, qt_sb[p]), (1, kt_sb[p])):
                    w8 = w8_sb[(p, side)]
                    bcol = bqk_sb[:, 4 * side + p:4 * side + p + 1]
                    for s0 in (0, 512):
                        pq = ps.tile([128, 512], F32, tag="small", bufs=2,
                                     name=f"pq{p}_{side}_{ch}_{s0}")
                        for k in range(CT):
                            nc.tensor.matmul(
                                pq,
                                lhsT=w8[:, k, :],
                                rhs=xt_sb[:, k,
                                          ch * CH + s0:ch * CH + s0 + 512],
                                start=(k == 0), stop=(k == CT - 1),
                            )
                        # q-side copies on DVE, k-side on ACT (GPSIMD cannot
                        # read PSUM): the first S matmul needs BOTH qt and kt
                        # — parallel engines halve that critical-path latency.
                        if side == 0:
                            nc.vector.tensor_scalar(
                                out=dst[:, ch * CH + s0:ch * CH + s0 + 512],
                                in0=pq, scalar1=bcol, scalar2=None,
                                op0=ALU.add,
                            )
                        else:
                            nc.scalar.activation(
                                out=dst[:, ch * CH + s0:ch * CH + s0 + 512],
                                in_=pq, func=AF.Identity, bias=bcol, scale=1.0,
                            )

            def emit_att(p, c):
                """Attention for pair p's two heads over query chunk c."""
                kmax = 8 * (c + 1)
                for hh in range(2):
                    hloc = 2 * p + hh
                    base = 64 * hh
                    qt_t, kt_t = qt_sb[p], kt_sb[p]
                    ytps = ps.tile([128, CH], F32, tag="ytps", bufs=1,
                                   name=f"ytps{hloc}_{c}")

                    def segs_of(ki):
                        q_off = max(0, 128 * ki - CH * c)
                        segs = []
                        if q_off < 512:
                            segs.append((q_off, 512))
                        segs.append((max(q_off, 512), CH))
                        return q_off, segs

                    def emit_s(ki):
                        q_off, segs = segs_of(ki)
                        stp = ps.tile([128, CH], F32, tag="stp", bufs=2,
                                      name=f"stp{hloc}_{c}_{ki}")
                        for (s0, s1) in segs:
                            nc.tensor.matmul(
                                stp[:, s0:s1],
                                lhsT=kt_t[base:base + 64,
                                          ki * 128:(ki + 1) * 128],
                                rhs=qt_t[base:base + 64,
                                         CH * c + s0:CH * c + s1],
                                start=True, stop=True,
                            )
                        pt = ptp.tile([128, CH], BF16, tag="pt",
                                      name=f"pt{hloc}_{c}_{ki}")
                        nc.scalar.activation(
                            out=pt[:, q_off:CH], in_=stp[:, q_off:CH],
                            func=AF.Exp, scale=SCALE,
                        )
                        if ki >= 8 * c:  # causal mask on diagonal block
                            nc.vector.tensor_mul(
                                pt[:, q_off:q_off + 128],
                                pt[:, q_off:q_off + 128], triu_sb,
                            )
                        return pt

                    b0_last = min(kmax - 1, 8 * c + 3)

                    def emit_av(ki, pt):
                        q_off, segs = segs_of(ki)
                        for (s0, s1) in segs:
                            last = b0_last if s0 < 512 else kmax - 1
                            nc.tensor.matmul(
                                ytps[:, s0:s1],
                                lhsT=v_sb[ki][:, 128 * hloc:128 * hloc + 128],
                                rhs=pt[:, s0:s1],
                                start=(ki == 0), stop=(ki == last),
                            )

                    # 2-deep software pipeline: S(0),S(1),S(2) precede
                    # AV(0), giving each chunk boundary ~2.5us of PE runway
                    # to cover the previous chunk's ytps drain chain (DVE
                    # rcp+mul, ~2.6us). Fillers pop where PE would wait for
                    # exp(ki-2).
                    pts = {0: emit_s(0)}
                    if kmax > 1:
                        pts[1] = emit_s(1)
                    for ki in range(2, kmax):
                        pts[ki] = emit_s(ki)
                        fill(1)
                        emit_av(ki - 2, pts.pop(ki - 2))
                    fill(1)
                    emit_av(kmax - 2, pts.pop(kmax - 2))
                    emit_av(kmax - 1, pts.pop(kmax - 1))

                    # normalize: y * (1/den). den sits on psum rows 64:127.
                    # Engine lanes are partition-locked: every compute op
                    # stays partition-aligned; the 64->0 partition move is an
                    # SBUF->SBUF DMA (engine-free, HW-verified pattern).
                    # Drain ytps through TWO engines in parallel — DVE
                    # reciprocal of den (rows 64:128) and ACT Identity copy
                    # of y (rows 0:64) — so ytps frees in ~1.3us (< the next
                    # head-chunk's S(0)+S(1) PE time); the DMA shift and the
                    # all-SBUF bf16 multiply trail off the critical path.
                    # DVE ops map lanes RELATIVELY within the partition
                    # range (the fp32r baseline did a cross-partition
                    # reciprocal on HW), so read den from rows 64:128 and
                    # write 1/den to rows 0:64 directly — no DMA shift, no
                    # ACT staging. Per column half so consumers (next
                    # head-chunk / out-proj) unblock sooner.
                    rcp_sb = rcpp.tile([64, CH], BF16, tag="rcp",
                                       name=f"rcp{hloc}_{c}")
                    for (d0, d1) in ((0, 512), (512, CH)):
                        with nc.allow_low_precision(
                            reason="1/denominator in bf16: 0.4% rel on a "
                                   "well-conditioned positive sum, budget 2e-2"
                        ):
                            nc.vector.reciprocal(
                                out=rcp_sb[:, d0:d1],
                                in_=ytps[64:128, d0:d1],
                            )
                        nc.vector.tensor_mul(
                            yt_sb[p][base:base + 64, CH * c + d0:CH * c + d1],
                            ytps[0:64, d0:d1], rcp_sb[:, d0:d1],
                        )

            def emit_out(qt_i, act_halves=(), split_dma=False):
                """Output projection for query tile qt_i + DMA to dram.

                act_halves: column halves whose psum->sbuf copy goes to the
                Activation engine — only safe once attention exp work there
                is done (ACT otherwise delays the exp stream).
                """
                drain_all()
                ot = outp.tile([128, C], BF16, tag="ot", name=f"ot{qt_i}")
                for s0 in (0, 512):
                    pso = ps.tile([128, 512], F32, tag="small", bufs=2,
                                  name=f"pso{qt_i}_{s0}")
                    for p in range(NPAIR):
                        nc.tensor.matmul(
                            pso,
                            lhsT=yt_sb[p][:, qt_i * 128:(qt_i + 1) * 128],
                            rhs=wp_sb[:, p, s0:s0 + 512],
                            start=(p == 0), stop=(p == NPAIR - 1),
                        )
                    if s0 in act_halves:
                        nc.scalar.activation(
                            out=ot[:, s0:s0 + 512], in_=pso,
                            func=AF.Copy, scale=1.0,
                        )
                    else:
                        nc.vector.tensor_copy(ot[:, s0:s0 + 512], pso)
                    if split_dma:  # final tiles: drain each half immediately
                        nc.sync.dma_start(
                            out=out.ap()[qt_i * 128:(qt_i + 1) * 128,
                                         s0:s0 + 512],
                            in_=ot[:, s0:s0 + 512],
                        )
                if not split_dma:
                    nc.sync.dma_start(
                        out=out.ap()[qt_i * 128:(qt_i + 1) * 128, :], in_=ot
                    )

            # ================= schedule =====================================
            for t in range(8):
                emit_v(t)
            for p in range(NPAIR):
                emit_qk(p, 0)
                # V tiles 8..15 become intra-attention fillers; pair 0 gets
                # none (its xt tail columns are still in flight on DMA).
                if p >= 1:
                    queue_v(6 + 2 * p)
                    queue_v(7 + 2 * p)
                emit_att(p, 0)
            queue_v(14)
            queue_v(15)
            while fill_q:  # V(14), V(15) + anything the slots didn't absorb
                fill(1)
            qk3_cnt = None
            for p in range(NPAIR):
                if p < NPAIR - 1:
                    emit_qk(p, 1)
                else:
                    # qk(3,ch1) was queued into att(2,c1); make sure every
                    # one of its ops is emitted before att(3,c1) reads qt/kt
                    while qk3_cnt["n"] > 0:
                        fill(1)
                # out-proj qtiles 0..7 (chunk-0 queries, ready since c0 pass)
                # become intra-attention fillers for the c1 pass; qk(3,ch1)
                # is queued (FIFO-first) into att(2,c1) instead of a block.
                if p == NPAIR - 2:
                    qk3_cnt = queue_qk(NPAIR - 1, 1)
                queue_out(2 * p)
                queue_out(2 * p + 1)
                emit_att(p, 1)
            while fill_q:
                fill(1)
            for qt_i in range(8, KT):
                # alternate whole-qtile copy engine so neither DVE nor ACT
                # serializes the tail
                halves = (0, 512) if qt_i % 2 else ()
                emit_out(qt_i, act_halves=halves, split_dma=True)

    nc.compile()
    _prog_cache[key] = nc
    return nc


def shard_inputs(x, W_qkv, b_qkv, W_proj, core):
    b, g = core // 2, core % 2
    cq = slice(512 * g, 512 * g + 512)
    ck = slice(1024 + 512 * g, 1024 + 512 * g + 512)
    cv = slice(2048 + 512 * g, 2048 + 512 * g + 512)
    return {
        "xt": np.ascontiguousarray(x[b].T).astype(NPBF16),
        "wq": np.ascontiguousarray(W_qkv[:, cq]).astype(NPBF16),
        "wk": np.ascontiguousarray(W_qkv[:, ck]).astype(NPBF16),
        "wv": np.ascontiguousarray(W_qkv[:, cv]).astype(NPBF16),
        "bqk_t": np.stack(
            [b_qkv[cq].reshape(4, 128)[p_] for p_ in range(4)]
            + [b_qkv[ck].reshape(4, 128)[p_] for p_ in range(4)], axis=1
        ).astype(np.float32).copy(),
        "bv": np.ascontiguousarray(b_qkv[cv]).reshape(1, 512).astype(NPBF16),
        "wp": np.ascontiguousarray(W_proj[512 * g:512 * g + 512, :]).astype(NPBF16),
    }


def kernel(x, W_qkv, b_qkv, W_proj, b_proj, **run_kwargs):
    x = np.asarray(x, np.float32)
    W_qkv = np.asarray(W_qkv, np.float32)
    b_qkv = np.asarray(b_qkv, np.float32)
    W_proj = np.asarray(W_proj, np.float32)
    b_proj = np.asarray(b_proj, np.float32)

    nc = build_program()
    in_maps = [
        shard_inputs(x, W_qkv, b_qkv, W_proj, core) for core in range(NCORES)
    ]
    from concourse.bass_utils import run_bass_kernel_spmd

    res = run_bass_kernel_spmd(nc, in_maps, core_ids=list(range(NCORES)), **run_kwargs)
    outs = [np.asarray(r["out"], np.float32) for r in res.results]
    full = np.stack([outs[2 * b_] + outs[2 * b_ + 1] + b_proj for b_ in range(B)])
    kernel.last_results = res
    return full


# revision 63
# speedup vs baseline: 1.0564x; 1.0069x over previous
"""Causal self-attention (B=4, T=2048, C=1024, H=16, D=64) on 8 TRN2 NeuronCores.

Sharding: core = 2*b + g  (b = batch 0..3, g = head-group 0..1; heads 8g..8g+7).
Each core computes, for its batch b and its 8 heads:
  qkv projection, causal softmax attention, and a PARTIAL output projection
  (its 512 rows of W_proj). Host sums the two partials per batch (+ b_proj).

Final design (301.7us baseline -> 262.6us; bf16 matmuls, f32 PSUM accum):
  - all matmul operands bf16: 1 cycle/row on PE at ANY free size (fp32r pays
    4x below 256 cols), halved DMA and SBUF footprint. bf16 noise ~3e-3 rel
    vs the 2e-2 budget (fp8 would blow it: ~5% q/k error -> ~0.2 abs).
  - softmax denominator for free: AV lhsT = [V_h | ones64] (128 cols), so
    PSUM rows 64:127 = sum_k P replicated across 64 partitions. Normalize =
    DVE reciprocal (partition-aligned 64:128) + ACT Identity copy of y,
    then SBUF->SBUF DMA shifts 1/den down to partitions 0:63 (engines are
    partition-locked; only DMA/PE can move data across partitions) + DVE
    multiply. Only ONE psum operand per vector op is legal.
  - consolidated DMAs: transfers run FIFO in descriptor order at ~350 B/ns
    with ~625ns serial descriptor gen per queue, so emission order IS the
    transfer schedule; front-load exactly what the V phase consumes first.
  - chunk-outer schedule with a filler queue: V(t0..7); {qk(p,ch0);
    att(p,c0) + V(t8..15) fillers} x4; {qk(p,ch1); att(p,c1) + out-proj
    qtile 0..7 fillers} x4; out-proj 8..15. The exp stream on ACT
    (0.83ns/col, ~152us) is the attention-phase co-bottleneck; popping one
    independent PE op between S(ki) and AV(ki-1) keeps PE fed through exp
    latency. Fillers and direct emitters share the "small" psum tag, so the
    queue must fully drain before any direct allocation (buffer rotation
    would corrupt an in-flight accumulation).
  - 1-deep software pipeline inside a head-chunk: emit S(ki+1) before AV(ki).
  - PE warmup: throwaway matmuls during the initial DMA wait ramp the PE
    p-state (1.54 -> 0.42 ns/col) before real work arrives.
  - PSUM banks (8): stp [128,1024] bufs=2 (4) + ytps [128,1024] (2) +
    small [128,512] bufs=2 (2).
  - psum->SBUF copy engines chosen to keep FIFOs clear: qk q-side DVE,
    k-side ACT Identity (per-partition bias AP); out-proj tail alternates
    DVE/ACT; V copies DVE; v-ones memsets per-tile on DVE (strided).
"""

import sys

try:
    import concourse  # noqa: F401
except ImportError:
    sys.path.insert(0, "/opt/trn_rl_repo")

import numpy as np
import ml_dtypes

import concourse.bacc as bacc
import concourse.mybir as mybir
import concourse.tile as tile

F32 = mybir.dt.float32
BF16 = mybir.dt.bfloat16
AF = mybir.ActivationFunctionType
ALU = mybir.AluOpType

B, T, C = 4, 2048, 1024
H, D = 16, 64
NCORES = 8
HL = 8          # heads per core (local)
NPAIR = 4       # head pairs per core
CH = 1024       # query chunk
NCH = T // CH   # 2
KT = T // 128   # 16 key tiles
CT = C // 128   # 8 contraction tiles over C
SCALE = 1.0 / 8.0  # 1/sqrt(D)

NPBF16 = ml_dtypes.bfloat16

_prog_cache = {}


def build_program(debug=False):
    key = debug
    if key in _prog_cache:
        return _prog_cache[key]

    nc = bacc.Bacc(None, target_bir_lowering=False, debug=debug)

    xt = nc.dram_tensor("xt", [C, T], BF16, kind="ExternalInput")
    wq = nc.dram_tensor("wq", [C, 512], BF16, kind="ExternalInput")
    wk = nc.dram_tensor("wk", [C, 512], BF16, kind="ExternalInput")
    wv = nc.dram_tensor("wv", [C, 512], BF16, kind="ExternalInput")
    bqk_t = nc.dram_tensor("bqk_t", [128, 8], F32, kind="ExternalInput")
    bv = nc.dram_tensor("bv", [1, 512], BF16, kind="ExternalInput")
    wp = nc.dram_tensor("wp", [512, C], BF16, kind="ExternalInput")
    out = nc.dram_tensor("out", [T, C], BF16, kind="ExternalOutput")

    with tile.TileContext(nc) as tc:
        with (
            tc.tile_pool(name="consts", bufs=1) as consts,
            tc.tile_pool(name="xtp", bufs=1) as xtp,
            tc.tile_pool(name="wvp", bufs=1) as wvp,
            tc.tile_pool(name="w8p", bufs=1) as w8p,
            tc.tile_pool(name="wpp", bufs=1) as wpp,
            tc.tile_pool(name="vp", bufs=1) as vp,
            tc.tile_pool(name="qkp", bufs=1) as qkp,
            tc.tile_pool(name="ytp", bufs=1) as ytp,
            tc.tile_pool(name="ptp", bufs=6) as ptp,
            tc.tile_pool(name="rcpp", bufs=2) as rcpp,
            tc.tile_pool(name="ytup", bufs=2) as ytup,
            tc.tile_pool(name="outp", bufs=3) as outp,
            tc.tile_pool(name="ps", bufs=1, space="PSUM") as ps,
        ):
            # ================= DMA staging (emission order = priority) ======
            # Transfers run FIFO in descriptor order on the shared DMA-engine
            # pool (~350 B/ns aggregate) and descriptor gen is ~625ns serial
            # per queue — so emission order IS the transfer schedule. One SP
            # chain, ordered by when the compute consumes each piece.
            xt_sb = xtp.tile([128, CT, T], BF16, tag="xt")
            xt_r = xt.ap().rearrange("(k p) t -> p k t", p=128)
            nc.sync.dma_start(out=xt_sb[:, :, 0:128], in_=xt_r[:, :, 0:128])
            wv_sb = wvp.tile([128, CT, 512], BF16, tag="wv")
            wv_r = wv.ap().rearrange("(k p) n -> p k n", p=128)
            for (k0_, k1_) in ((0, 1), (1, 4), (4, CT)):
                nc.sync.dma_start(
                    out=wv_sb[:, k0_:k1_, :], in_=wv_r[:, k0_:k1_, :]
                )
            vecs = consts.tile([128, 640], BF16, tag="vecs")
            bv_sb = vecs[32:33, 0:512]
            nc.sync.dma_start(out=bv_sb, in_=bv[:, :])
            for (c0_, c1_) in ((128, 256), (256, 512), (512, 1024)):
                nc.sync.dma_start(
                    out=xt_sb[:, :, c0_:c1_], in_=xt_r[:, :, c0_:c1_]
                )

            bqk_sb = consts.tile([128, 8], F32, tag="bqk")
            nc.sync.dma_start(out=bqk_sb, in_=bqk_t[:, :])

            # qk weights: w8[(p, side)] = [128, CT, 128] bf16 (one desc each)
            w8_sb = {}
            for p in range(NPAIR):
                for side, wsrc in ((0, wq), (1, wk)):
                    t_ = w8p.tile([128, CT, 128], BF16, tag=f"w8_{p}_{side}")
                    w_src = wsrc.ap().rearrange(
                        "(k pp) m -> pp k m", pp=128
                    )[:, :, p * 128:(p + 1) * 128]
                    nc.sync.dma_start(out=t_, in_=w_src)
                    w8_sb[(p, side)] = t_

            # wp: one descriptor, [128, NPAIR, 1024]
            wp_sb = wpp.tile([128, NPAIR, C], BF16, tag="wp")
            nc.sync.dma_start(
                out=wp_sb, in_=wp.ap().rearrange("(k p) n -> p k n", p=128)
            )
            # xt tail columns — only needed from V(t8..15) / qk ch1 onward
            for (c0_, c1_) in ((1024, 1536), (1536, 2048)):
                nc.sync.dma_start(
                    out=xt_sb[:, :, c0_:c1_], in_=xt_r[:, :, c0_:c1_]
                )

            # constants
            ones_f32 = consts.tile([128, 128], F32, tag="ones_f32")
            nc.vector.memset(ones_f32, 1.0)
            triu_f32 = consts.tile([128, 128], F32, tag="triu_f32")
            nc.gpsimd.memset(triu_f32, 1.0)
            nc.gpsimd.affine_select(
                out=triu_f32, in_=triu_f32,
                compare_op=ALU.is_ge,
                fill=0.0, base=0, pattern=[[1, 128]], channel_multiplier=-1,
            )
            triu_sb = consts.tile([128, 128], BF16, tag="triu")
            nc.vector.tensor_copy(triu_sb, triu_f32)
            # rank-1 v-bias vector ones: lhsT/rhs share base partition 32
            ones32_sb = vecs[32:33, 512:640]
            nc.vector.tensor_copy(ones32_sb, ones_f32[0:1, 0:128])

            # ================= persistent SBUF state ========================
            # v tiles: [128, 8 heads x (64 v-dims | 64 ones)]
            v_sb = []
            for t in range(KT):
                t_ = vp.tile([128, HL * 128], BF16, tag=f"v{t}", name=f"v{t}")
                v_sb.append(t_)

            def memset_v_ones(t):
                # ones columns only (strided): DVE, emitted per-tile right
                # before its V projection so 16 memsets don't jam the DVE
                # FIFO ahead of the V-phase psum copies. (gpsimd.memset on a
                # bf16 tile produced wrong bits on HW — keep this on DVE.)
                v_r = v_sb[t].rearrange("p (h x) -> p h x", h=HL)
                nc.vector.memset(v_r[:, :, 64:128], 1.0)
            qt_sb = [qkp.tile([128, T], BF16, tag=f"qt{p}", name=f"qt{p}")
                     for p in range(NPAIR)]
            kt_sb = [qkp.tile([128, T], BF16, tag=f"kt{p}", name=f"kt{p}")
                     for p in range(NPAIR)]
            yt_sb = [ytp.tile([128, T], BF16, tag=f"yt{p}", name=f"yt{p}")
                     for p in range(NPAIR)]

            # ---- PE warmup: the tensor engine ramps (1.54 -> 0.83 -> 0.42
            # ns/col over ~3us of continuous work). Burn the initial DMA wait
            # on throwaway matmuls over the freshly-memset v tile so the real
            # V projection starts at full clock.
            warm = ps.tile([128, CH], F32, tag="stp", bufs=2, name="warm")
            for i_ in range(12):
                s0 = 512 * (i_ % 2)
                nc.tensor.matmul(
                    warm[:, s0:s0 + 512],
                    lhsT=v_sb[0][:, 0:128], rhs=v_sb[0][:, 0:512],
                    start=True, stop=True,
                )

            # ================= filler queue =================================
            # The exp stream on ACT runs ~0.2us/ki slower than PE's S+AV, so
            # stalls accumulate inside a head-chunk. Queue independent PE work
            # (V tiles, out-proj qtiles) as single-matmul callables and pop
            # one between S(ki) and AV(ki-1) — PE chews filler exactly where
            # it would otherwise wait for exp(ki-1).
            from collections import deque
            fill_q = deque()

            def fill(n=1):
                for _ in range(n):
                    if not fill_q:
                        return
                    fill_q.popleft()()

            def drain_all():
                """MUST run before any direct 'small'/'ot' tile allocation:
                a queued unit left half-emitted would have its rotating psum
                buffer stolen mid-accumulation (silent corruption)."""
                while fill_q:
                    fill(1)

            def queue_v(t):
                """Enqueue V projection for key-tile t as per-op callables."""
                st = {}

                def mk_mm(k):
                    def f():
                        if k == 0:
                            st["pv"] = ps.tile([128, 512], F32, tag="small",
                                               bufs=2, name=f"pv{t}")
                        nc.tensor.matmul(
                            st["pv"],
                            lhsT=xt_sb[:, k, t * 128:(t + 1) * 128],
                            rhs=wv_sb[:, k, :], start=(k == 0), stop=False,
                        )
                    return f

                def bias():
                    nc.tensor.matmul(
                        st["pv"], lhsT=ones32_sb, rhs=bv_sb,
                        start=False, stop=True,
                    )

                def copy():
                    v_r = v_sb[t].rearrange("p (h x) -> p h x", h=HL)
                    pv_r = st["pv"].rearrange("p (h d) -> p h d", h=HL)
                    nc.vector.tensor_copy(v_r[:, :, 0:64], pv_r)

                fill_q.append(lambda: memset_v_ones(t))
                for k in range(CT):
                    fill_q.append(mk_mm(k))
                fill_q.append(bias)
                fill_q.append(copy)

            def queue_qk(p, ch):
                """Enqueue qk projection (filler variant: copies on DVE only,
                so no exp-stream interference when popped mid-attention).
                Returns a counter dict; drain until counter hits 0 before
                emitting anything that reads qt/kt of this pair+chunk."""
                st = {}
                cnt = {"n": 0}

                def wrap(f):
                    cnt["n"] += 1

                    def g():
                        f()
                        cnt["n"] -= 1
                    return g

                def mk_mm(side, s0, k):
                    def f():
                        if k == 0:
                            st[(side, s0)] = ps.tile(
                                [128, 512], F32, tag="small", bufs=2,
                                name=f"pq{p}_{side}_{ch}_{s0}")
                        nc.tensor.matmul(
                            st[(side, s0)],
                            lhsT=w8_sb[(p, side)][:, k, :],
                            rhs=xt_sb[:, k, ch * CH + s0:ch * CH + s0 + 512],
                            start=(k == 0), stop=(k == CT - 1),
                        )
                    return f

                def mk_copy(side, s0):
                    dst = qt_sb[p] if side == 0 else kt_sb[p]
                    bcol = bqk_sb[:, 4 * side + p:4 * side + p + 1]

                    def f():
                        nc.vector.tensor_scalar(
                            out=dst[:, ch * CH + s0:ch * CH + s0 + 512],
                            in0=st[(side, s0)], scalar1=bcol, scalar2=None,
                            op0=ALU.add,
                        )
                    return f

                for side in (0, 1):
                    for s0 in (0, 512):
                        for k in range(CT):
                            fill_q.append(wrap(mk_mm(side, s0, k)))
                        fill_q.append(wrap(mk_copy(side, s0)))
                return cnt

            def queue_out(qt_i):
                """Enqueue out-projection for query tile qt_i (DVE copies)."""
                st = {}

                def mk_mm(s0, p):
                    def f():
                        if p == 0:
                            st[s0] = ps.tile([128, 512], F32, tag="small",
                                             bufs=2, name=f"pso{qt_i}_{s0}")
                        nc.tensor.matmul(
                            st[s0],
                            lhsT=yt_sb[p][:, qt_i * 128:(qt_i + 1) * 128],
                            rhs=wp_sb[:, p, s0:s0 + 512],
                            start=(p == 0), stop=(p == NPAIR - 1),
                        )
                    return f

                def mk_copy(s0):
                    def f():
                        if "ot" not in st:
                            st["ot"] = outp.tile([128, C], BF16, tag="ot",
                                                 name=f"ot{qt_i}")
                        nc.vector.tensor_copy(
                            st["ot"][:, s0:s0 + 512], st[s0]
                        )
                    return f

                def dma():
                    nc.sync.dma_start(
                        out=out.ap()[qt_i * 128:(qt_i + 1) * 128, :],
                        in_=st["ot"],
                    )

                for s0 in (0, 512):
                    for p in range(NPAIR):
                        fill_q.append(mk_mm(s0, p))
                    fill_q.append(mk_copy(s0))
                fill_q.append(dma)

            # ================= phase emitters ===============================
            def emit_v(t):
                """V projection for key-tile t -> v_sb[t]."""
                memset_v_ones(t)
                pv = ps.tile([128, 512], F32, tag="small", bufs=2,
                             name=f"pv{t}")
                for k in range(CT):
                    nc.tensor.matmul(
                        pv,
                        lhsT=xt_sb[:, k, t * 128:(t + 1) * 128],
                        rhs=wv_sb[:, k, :], start=(k == 0), stop=False,
                    )
                nc.tensor.matmul(
                    pv, lhsT=ones32_sb, rhs=bv_sb,
                    start=False, stop=True,
                )
                v_r = v_sb[t].rearrange("p (h x) -> p h x", h=HL)
                pv_r = pv.rearrange("p (h d) -> p h d", h=HL)
                nc.vector.tensor_copy(v_r[:, :, 0:64], pv_r)

            def emit_qk(p, ch):
                """qk projection for pair p, T-chunk ch -> qt/kt cols."""
                drain_all()
                for side, dst in ((0, qt_sb[p]), (1, kt_sb[p])):
                    w8 = w8_sb[(p, side)]
                    bcol = bqk_sb[:, 4 * side + p:4 * side + p + 1]
                    for s0 in (0, 512):
                        pq = ps.tile([128, 512], F32, tag="small", bufs=2,
                                     name=f"pq{p}_{side}_{ch}_{s0}")
                        for k in range(CT):
                            nc.tensor.matmul(
                                pq,
                                lhsT=w8[:, k, :],
                                rhs=xt_sb[:, k,
                                          ch * CH + s0:ch * CH + s0 + 512],
                                start=(k == 0), stop=(k == CT - 1),
                            )
                        # q-side copies on DVE, k-side on ACT (GPSIMD cannot
                        # read PSUM): the first S matmul needs BOTH qt and kt
                        # — parallel engines halve that critical-path latency.
                        if side == 0:
                            nc.vector.tensor_scalar(
                                out=dst[:, ch * CH + s0:ch * CH + s0 + 512],
                                in0=pq, scalar1=bcol, scalar2=None,
                                op0=ALU.add,
                            )
                        else:
                            nc.scalar.activation(
                                out=dst[:, ch * CH + s0:ch * CH + s0 + 512],
                                in_=pq, func=AF.Identity, bias=bcol, scale=1.0,
                            )

            def emit_att(p, c):
                """Attention for pair p's two heads over query chunk c."""
                kmax = 8 * (c + 1)
                for hh in range(2):
                    hloc = 2 * p + hh
                    base = 64 * hh
                    qt_t, kt_t = qt_sb[p], kt_sb[p]
                    ytps = ps.tile([128, CH], F32, tag="ytps", bufs=1,
                                   name=f"ytps{hloc}_{c}")

                    def segs_of(ki):
                        q_off = max(0, 128 * ki - CH * c)
                        segs = []
                        if q_off < 512:
                            segs.append((q_off, 512))
                        segs.append((max(q_off, 512), CH))
                        return q_off, segs

                    def emit_s(ki):
                        q_off, segs = segs_of(ki)
                        stp = ps.tile([128, CH], F32, tag="stp", bufs=2,
                                      name=f"stp{hloc}_{c}_{ki}")
                        for (s0, s1) in segs:
                            nc.tensor.matmul(
                                stp[:, s0:s1],
                                lhsT=kt_t[base:base + 64,
                                          ki * 128:(ki + 1) * 128],
                                rhs=qt_t[base:base + 64,
                                         CH * c + s0:CH * c + s1],
                                start=True, stop=True,
                            )
                        pt = ptp.tile([128, CH], BF16, tag="pt",
                                      name=f"pt{hloc}_{c}_{ki}")
                        nc.scalar.activation(
                            out=pt[:, q_off:CH], in_=stp[:, q_off:CH],
                            func=AF.Exp, scale=SCALE,
                        )
                        if ki >= 8 * c:  # causal mask on diagonal block
                            nc.vector.tensor_mul(
                                pt[:, q_off:q_off + 128],
                                pt[:, q_off:q_off + 128], triu_sb,
                            )
                        return pt

                    b0_last = min(kmax - 1, 8 * c + 3)

                    def emit_av(ki, pt):
                        q_off, segs = segs_of(ki)
                        for (s0, s1) in segs:
                            last = b0_last if s0 < 512 else kmax - 1
                            nc.tensor.matmul(
                                ytps[:, s0:s1],
                                lhsT=v_sb[ki][:, 128 * hloc:128 * hloc + 128],
                                rhs=pt[:, s0:s1],
                                start=(ki == 0), stop=(ki == last),
                            )

                    # 2-deep software pipeline: S(0),S(1),S(2) precede
                    # AV(0), giving each chunk boundary ~2.5us of PE runway
                    # to cover the previous chunk's ytps drain chain (DVE
                    # rcp+mul, ~2.6us). Fillers pop where PE would wait for
                    # exp(ki-2).
                    pts = {0: emit_s(0)}
                    if kmax > 1:
                        pts[1] = emit_s(1)
                    for ki in range(2, kmax):
                        pts[ki] = emit_s(ki)
                        fill(1)
                        emit_av(ki - 2, pts.pop(ki - 2))
                    fill(1)
                    emit_av(kmax - 2, pts.pop(kmax - 2))
                    emit_av(kmax - 1, pts.pop(kmax - 1))

                    # normalize: y * (1/den). den sits on psum rows 64:127.
                    # Engine lanes are partition-locked: every compute op
                    # stays partition-aligned; the 64->0 partition move is an
                    # SBUF->SBUF DMA (engine-free, HW-verified pattern).
                    # Drain ytps through TWO engines in parallel — DVE
                    # reciprocal of den (rows 64:128) and ACT Identity copy
                    # of y (rows 0:64) — so ytps frees in ~1.3us (< the next
                    # head-chunk's S(0)+S(1) PE time); the DMA shift and the
                    # all-SBUF bf16 multiply trail off the critical path.
                    # DVE ops map lanes RELATIVELY within the partition
                    # range (the fp32r baseline did a cross-partition
                    # reciprocal on HW), so read den from rows 64:128 and
                    # write 1/den to rows 0:64 directly — no DMA shift, no
                    # ACT staging. Per column half so consumers (next
                    # head-chunk / out-proj) unblock sooner.
                    rcp_sb = rcpp.tile([64, CH], BF16, tag="rcp",
                                       name=f"rcp{hloc}_{c}")
                    if p == NPAIR - 1 and c == 1 and hh == 1:
                        # last head-chunk: quarter granularity so the tail
                        # out-projection's pair-3 matmuls unblock asap
                        nspans = tuple((q * 256, q * 256 + 256)
                                       for q in range(4))
                    else:
                        nspans = ((0, 512), (512, CH))
                    for (d0, d1) in nspans:
                        with nc.allow_low_precision(
                            reason="1/denominator in bf16: 0.4% rel on a "
                                   "well-conditioned positive sum, budget 2e-2"
                        ):
                            nc.vector.reciprocal(
                                out=rcp_sb[:, d0:d1],
                                in_=ytps[64:128, d0:d1],
                            )
                        nc.vector.tensor_mul(
                            yt_sb[p][base:base + 64, CH * c + d0:CH * c + d1],
                            ytps[0:64, d0:d1], rcp_sb[:, d0:d1],
                        )

            def emit_out(qt_i, act_halves=(), split_dma=False):
                """Output projection for query tile qt_i + DMA to dram.

                act_halves: column halves whose psum->sbuf copy goes to the
                Activation engine — only safe once attention exp work there
                is done (ACT otherwise delays the exp stream).
                """
                drain_all()
                ot = outp.tile([128, C], BF16, tag="ot", name=f"ot{qt_i}")
                for s0 in (0, 512):
                    pso = ps.tile([128, 512], F32, tag="small", bufs=2,
                                  name=f"pso{qt_i}_{s0}")
                    for p in range(NPAIR):
                        nc.tensor.matmul(
                            pso,
                            lhsT=yt_sb[p][:, qt_i * 128:(qt_i + 1) * 128],
                            rhs=wp_sb[:, p, s0:s0 + 512],
                            start=(p == 0), stop=(p == NPAIR - 1),
                        )
                    if s0 in act_halves:
                        nc.scalar.activation(
                            out=ot[:, s0:s0 + 512], in_=pso,
                            func=AF.Copy, scale=1.0,
                        )
                    else:
                        nc.vector.tensor_copy(ot[:, s0:s0 + 512], pso)
                    if split_dma:  # final tiles: drain each half immediately
                        nc.sync.dma_start(
                            out=out.ap()[qt_i * 128:(qt_i + 1) * 128,
                                         s0:s0 + 512],
                            in_=ot[:, s0:s0 + 512],
                        )
                if not split_dma:
                    nc.sync.dma_start(
                        out=out.ap()[qt_i * 128:(qt_i + 1) * 128, :], in_=ot
                    )

            # ================= schedule =====================================
            for t in range(8):
                emit_v(t)
            for p in range(NPAIR):
                emit_qk(p, 0)
                # V tiles 8..15 become intra-attention fillers; pair 0 gets
                # none (its xt tail columns are still in flight on DMA).
                if p >= 1:
                    queue_v(6 + 2 * p)
                    queue_v(7 + 2 * p)
                emit_att(p, 0)
            queue_v(14)
            queue_v(15)
            while fill_q:  # V(14), V(15) + anything the slots didn't absorb
                fill(1)
            qk3_cnt = None
            for p in range(NPAIR):
                if p < NPAIR - 1:
                    emit_qk(p, 1)
                else:
                    # qk(3,ch1) was queued into att(2,c1); make sure every
                    # one of its ops is emitted before att(3,c1) reads qt/kt
                    while qk3_cnt["n"] > 0:
                        fill(1)
                # out-proj qtiles 0..7 (chunk-0 queries, ready since c0 pass)
                # become intra-attention fillers for the c1 pass; qk(3,ch1)
                # is queued (FIFO-first) into att(2,c1) instead of a block.
                if p == NPAIR - 2:
                    qk3_cnt = queue_qk(NPAIR - 1, 1)
                queue_out(2 * p)
                queue_out(2 * p + 1)
                emit_att(p, 1)
            while fill_q:
                fill(1)
            for qt_i in range(8, KT):
                # alternate whole-qtile copy engine so neither DVE nor ACT
                # serializes the tail
                halves = (0, 512) if qt_i % 2 else ()
                emit_out(qt_i, act_halves=halves, split_dma=True)

    nc.compile()
    _prog_cache[key] = nc
    return nc


def shard_inputs(x, W_qkv, b_qkv, W_proj, core):
    b, g = core // 2, core % 2
    cq = slice(512 * g, 512 * g + 512)
    ck = slice(1024 + 512 * g, 1024 + 512 * g + 512)
    cv = slice(2048 + 512 * g, 2048 + 512 * g + 512)
    return {
        "xt": np.ascontiguousarray(x[b].T).astype(NPBF16),
        "wq": np.ascontiguousarray(W_qkv[:, cq]).astype(NPBF16),
        "wk": np.ascontiguousarray(W_qkv[:, ck]).astype(NPBF16),
        "wv": np.ascontiguousarray(W_qkv[:, cv]).astype(NPBF16),
        "bqk_t": np.stack(
            [b_qkv[cq].reshape(4, 128)[p_] for p_ in range(4)]
            + [b_qkv[ck].reshape(4, 128)[p_] for p_ in range(4)], axis=1
        ).astype(np.float32).copy(),
        "bv": np.ascontiguousarray(b_qkv[cv]).reshape(1, 512).astype(NPBF16),
        "wp": np.ascontiguousarray(W_proj[512 * g:512 * g + 512, :]).astype(NPBF16),
    }


def kernel(x, W_qkv, b_qkv, W_proj, b_proj, **run_kwargs):
    x = np.asarray(x, np.float32)
    W_qkv = np.asarray(W_qkv, np.float32)
    b_qkv = np.asarray(b_qkv, np.float32)
    W_proj = np.asarray(W_proj, np.float32)
    b_proj = np.asarray(b_proj, np.float32)

    nc = build_program()
    in_maps = [
        shard_inputs(x, W_qkv, b_qkv, W_proj, core) for core in range(NCORES)
    ]
    from concourse.bass_utils import run_bass_kernel_spmd

    res = run_bass_kernel_spmd(nc, in_maps, core_ids=list(range(NCORES)), **run_kwargs)
    outs = [np.asarray(r["out"], np.float32) for r in res.results]
    full = np.stack([outs[2 * b_] + outs[2 * b_ + 1] + b_proj for b_ in range(B)])
    kernel.last_results = res
    return full
